# revision 4
# baseline (speedup 1.0000x reference)
"""GQA multi-head attention (B=2, S=2048, D=2048, 32 q-heads / 8 kv-heads)
on 8 Trainium2 NeuronCores.

Sharding: batch (2-way) x kv-head-pair (4-way). Core c owns batch c//4 and
kv heads {2g, 2g+1} with g = c%4, i.e. q heads 8g..8g+7 (512 q-proj feats).
Wq/Wk/Wv column-sharded, Wo row-sharded; host sums the 4 partials per batch.

Per-core dataflow (bf16 operands, fp32 PSUM):
  K^T [128f, S]  = Wk_c^T @ k^T          (2 kv heads fill all 128 partitions)
  V   [S, 128f]  = v-tiles^T... via lhsT=vT chunk, rhs=Wv_c (tokens on
                   partitions; +ones cols for softmax denominators)
  Q^T [128, 4, S]: ftile f rows u*64:(u+1)*64 = head (u,f) (u = kv half)
  S^T [kv, q] = ktd[u-half].T-slice @ qtp[u-half, f]   (64-contraction)
  expS = exp(S^T * 1/8) on ACT -> bf16 SBUF
  ctx [q, 65] = expS^T-chunk.T @ V_aug   (ones col -> rowsum at col 64)
  cnorm = ctx[:,0:64] * recip(ctx[:,64]) (DVE per-partition scalar mul)
  ctxT via PE transpose; out [q, dm] = ctxT.T @ Wo_c -> DMA fp32 psum->HBM
"""

from contextlib import ExitStack

import numpy as np
import ml_dtypes

import jax

try:
    jax.config.update("jax_compilation_cache_dir", "/tmp/jax_bass_cache")
    jax.config.update("jax_persistent_cache_min_compile_time_secs", 1.0)
except Exception:
    pass

from jax.sharding import Mesh, PartitionSpec, NamedSharding
from jax.experimental.shard_map import shard_map

import concourse.bass as bass
import concourse.mybir as mybir
import concourse.tile as tile
from concourse import bacc, bass2jax

BF16 = mybir.dt.bfloat16
F32 = mybir.dt.float32
AF = mybir.ActivationFunctionType

B, S, DM = 2, 2048, 2048
NC = 8
DT = DM // 128         # 16 contraction tiles
DH = 64
DQ = 512               # per-core q-proj width (8 heads)
DKV = 128              # per-core kv width (2 heads)
NKT = S // 128         # 16 kv tiles
SCALE = 1.0 / 8.0      # 1/sqrt(64)

_cache = {}


def _emit(ctx, tc, qT, kT, vT, wq, wk, wv, wo, out):
    nc = tc.nc

    pp = ctx.enter_context(tc.tile_pool(name="persist", bufs=1))
    wq_sb = pp.tile([128, 4, DT, 128], BF16, tag="wq")   # [p, f, dt, d]
    wk_sb = pp.tile([128, DT, DKV], BF16, tag="wk")
    wv_sb = pp.tile([128, DT, DKV], BF16, tag="wv")
    wo_sb = pp.tile([128, 4, DM], BF16, tag="wo")
    qtp = pp.tile([128, 4, S], BF16, tag="qtp")      # [u*64+d, f, q]
    ktd = pp.tile([128, S], BF16, tag="ktd")         # [u*64+d, kv]
    vsb = pp.tile([128, NKT, 132], BF16, tag="vsb")  # [kv%128, kvtile, 2x66]
    ident = pp.tile([128, 128], BF16, tag="ident")

    from concourse.masks import make_identity
    make_identity(nc, ident[:])
    nc.gpsimd.memset(vsb[:, :, DH], 1.0)        # ones col, kv head 0
    nc.gpsimd.memset(vsb[:, :, 66 + DH], 1.0)   # ones col, kv head 1
    warm = pp.tile([128, 128], BF16, tag="warm")  # also feeds warm_fill

    stage = ctx.enter_context(tc.tile_pool(name="stage", bufs=3))
    expp = ctx.enter_context(tc.tile_pool(name="expp", bufs=4))
    smal = ctx.enter_context(tc.tile_pool(name="small", bufs=3))
    ctxp = ctx.enter_context(tc.tile_pool(name="ctxp", bufs=2))
    psum = ctx.enter_context(tc.tile_pool(name="psum", bufs=1, space="PSUM"))

    # ---------------- staged input DMA helpers ----------------
    def dma_k(c):
        k_ch = stage.tile([128, DT, 512], BF16, tag="instage", name=f"k_ch_{c}")
        nc.sync.dma_start(
            k_ch[:],
            kT.rearrange("(dt p) s -> p dt s", p=128)[:, :, c * 512:(c + 1) * 512])
        return k_ch

    def dma_q(qc):
        q_ch = stage.tile([128, DT, 512], BF16, tag="instage", name=f"q_ch_{qc}")
        nc.sync.dma_start(
            q_ch[:],
            qT.rearrange("(dt p) s -> p dt s", p=128)[:, :, qc * 512:(qc + 1) * 512])
        return q_ch

    def dma_v(c):
        v_ch = stage.tile([128, DT, 512], BF16, tag="instage", name=f"v_ch_{c}")
        nc.sync.dma_start(
            v_ch[:],
            vT.rearrange("(dt p) s -> p dt s", p=128)[:, :, c * 512:(c + 1) * 512])
        return v_ch

    # Weights arrive pre-arranged on the host so every DMA has >=512B
    # contiguous runs (full DMA rate). K streams in 5 chunks, narrow first,
    # so K-proj and the first score pairs start as early as possible.
    K_CH = [(0, 256), (256, 256), (512, 512), (1024, 512), (1536, 512)]

    def dma_kc(i):
        off, w = K_CH[i]
        k_ch = stage.tile([128, DT, w], BF16, tag="instage", name=f"k_ch_{i}")
        nc.sync.dma_start(
            k_ch[:],
            kT.rearrange("(dt p) s -> p dt s", p=128)[:, :, off:off + w])
        return k_ch

    nc.sync.dma_start(wk_sb[:], wk.rearrange("p (dt m) -> p dt m", m=DKV))
    k_chs = [dma_kc(0)]
    wqr = wq.rearrange("p (f dt m) -> p f dt m", f=4, m=128)
    nc.sync.dma_start(wq_sb[:, 0], wqr[:, 0])
    q_ch0 = dma_q(0)
    k_chs.append(dma_kc(1))
    nc.sync.dma_start(wq_sb[:, 1:4], wqr[:, 1:4])
    nc.sync.dma_start(wv_sb[:], wv.rearrange("p (dt m) -> p dt m", m=DKV))
    k_chs += [dma_kc(2), dma_kc(3), dma_kc(4)]
    v_chs = [dma_v(c) for c in range(4)]

    def k_proj(c):
        off, w = K_CH[c]
        pk = psum.tile([128, w], F32, tag="mm", bufs=2, name=f"pk_{c}")
        for dt in range(DT):
            nc.tensor.matmul(pk[:], wk_sb[:, dt, :], k_chs[c][:, dt, :],
                             start=(dt == 0), stop=(dt == DT - 1))
        nc.vector.tensor_copy(ktd[:, off:off + w], pk[:])

    def v_proj(t):
        # demoted priority: these chains unblock when their vT chunk lands
        # mid-attention; without the demotion they preempt the score pairs
        # that feed ACT (priority inversion -> ACT starvation)
        with tc.high_priority(-400):
            pv = psum.tile([128, DKV], F32, tag="mm", bufs=2, name=f"pv_{t}")
            for dt in range(DT):
                nc.tensor.matmul(
                    pv[:], v_chs[t // 4][:, dt, (t % 4) * 128:(t % 4 + 1) * 128],
                    wv_sb[:, dt, :], start=(dt == 0), stop=(dt == DT - 1))
            # [:, 0:64] -> cols 0:64 (head 0), [:, 64:128] -> cols 66:130
            dst = vsb[:, t, :].rearrange("p (a b) -> p a b", b=66)[:, :, 0:DH]
            nc.vector.tensor_copy(dst, pv[:].rearrange("p (a b) -> p a b", b=DH))

    def q_chain_block(pq, q_ch, f, quarter):
        """4 of the 16 accumulation steps of Q-proj chain f."""
        for dt in range(quarter * 4, quarter * 4 + 4):
            nc.tensor.matmul(
                pq[:], wq_sb[:, f, dt, :], q_ch[:, dt, :],
                start=(dt == 0), stop=(dt == DT - 1))

    from contextlib import contextmanager

    @contextmanager
    def _null():
        yield

    def q_proj_one(qc, q_ch, f, demote=0):
        with tc.high_priority(-demote) if demote else _null():
            pq = psum.tile([128, 512], F32, tag="mm", bufs=2,
                           name=f"pq_{qc}_{f}")
            for quarter in range(4):
                q_chain_block(pq, q_ch, f, quarter)
            nc.vector.tensor_copy(qtp[:, f, qc * 512:(qc + 1) * 512], pq[:])

    # Warm-up matmuls: keep the PE busy (and its p-state ramping toward full
    # clock) while the first input DMAs stream in. Results are never read.
    nc.vector.memset(warm[:], 0.0)

    def warmup(n):
        # Keep the PE continuously busy (matmul p-state stays ramped) while
        # input DMAs stream in; results are never read. Priority order lets
        # real work preempt the remaining warmups the moment it is ready.
        for w in range(n):
            pw = psum.tile([128, 2, 128], F32, tag="sc", bufs=2,
                           name=f"warm_{warmup.i}")
            warmup.i += 1
            nc.tensor.matmul(pw[:, 0], warm[:], warm[:])
            nc.tensor.matmul(pw[:, 1], warm[:], warm[:])
    warmup.i = 0

    # PE preamble: narrow K chunks and Q proj(0) interleaved with arrival
    # order; V proj chains are deferred into the qc0 filler stream. Warmups
    # interleave so the PE never idles (a PE idle gap resets the clock ramp
    # and the next chain would dispatch at the cold rate).
    warmup(25)
    k_proj(0)
    warmup(34)
    q_proj_one(0, q_ch0, 0)
    k_proj(1)
    for f in range(1, 4):
        q_proj_one(0, q_ch0, f)
    for c in range(2, 5):
        k_proj(c)

    # ---------------- pipelined attention + output projection ----------------
    # Score-pair emissions are the clock; between pairs we drain ~PAIR_NS of
    # deferred PE micro-tasks (ctx chains, transposes, out-proj chunks,
    # quarter Q-proj chains, V-proj chains) from a FIFO.
    from collections import deque
    fillers = deque()   # (cost_ns, thunk, tag)
    PAIR_NS = 700.0

    def drain(budget):
        got = 0.0
        while fillers and got < budget:
            cost, thunk, _ = fillers.popleft()
            thunk()
            got += cost

    def drain_tag(tag):
        """Emit (FIFO) until no filler with `tag` remains — order barrier."""
        while any(t == tag for _, _, t in fillers):
            cost, thunk, _ = fillers.popleft()
            thunk()

    def pair(qc, hd, kt2, expt):
        u, f = hd // 4, hd % 4
        lo, hi = u * DH, (u + 1) * DH
        ps = psum.tile([128, 2, 512], F32, tag="sc", bufs=2,
                       name=f"ps_{qc}_{hd}_{kt2}")
        for j in range(2):
            kt = 2 * kt2 + j
            nc.tensor.matmul(
                ps[:, j, :],
                ktd[lo:hi, kt * 128:(kt + 1) * 128],
                qtp[lo:hi, f, qc * 512:(qc + 1) * 512])
        nc.scalar.activation(
            expt[:, 2 * kt2:2 * kt2 + 2, :], ps[:], AF.Exp, scale=SCALE)

    # cn tiles buffer the normalized ctx of BOTH kv halves of an ftile
    # ([128 q, 128] = head f in cols 0:64, head f+4 in 64:128) so a single
    # XBAR DMA transpose produces the full-partition ctxT block — no PE
    # transpose, no DVE psum->sbuf copy.
    cns = {}

    def ctx_sub(qc, hd, sub, expt, cT):
        u, f = hd // 4, hd % 4

        def chain():
            pc = psum.tile([128, DH + 1], F32, tag="cx", bufs=2,
                           name=f"pc_{qc}_{hd}_{sub}")
            for kt in range(NKT):
                nc.tensor.matmul(
                    pc[:], expt[:, kt, sub * 128:(sub + 1) * 128],
                    vsb[:, kt, u * 66:u * 66 + DH + 1],
                    start=(kt == 0), stop=(kt == NKT - 1))
            rc = smal.tile([128, 1], F32, tag="rc", name=f"rc_{qc}_{hd}_{sub}")
            nc.vector.reciprocal(rc[:], pc[:, DH:DH + 1])
            if (qc, f, sub) not in cns:
                cns[(qc, f, sub)] = smal.tile(
                    [128, 128], BF16, tag="cn", bufs=20,
                    name=f"cn_{qc}_{f}_{sub}")
            cn = cns[(qc, f, sub)]
            nc.vector.tensor_scalar_mul(
                cn[:, u * DH:(u + 1) * DH], pc[:, 0:DH], rc[:])
            if u == 1:
                del cns[(qc, f, sub)]

                def transp():
                    nc.sync.dma_start_transpose(
                        cT[:, f, sub * 128:(sub + 1) * 128], cn[:])

                # near the FRONT (slightly deferred): must stay ahead of any
                # out-proj filler that reads this ctxT subtile.
                fillers.insert(min(2, len(fillers)),
                               (60.0, transp, f"ctx{qc}"))

        return chain

    def out_chunk(qc, sub, ch, cT, ost, tail=False):
        po = psum.tile([128, 512], F32, tag="mm", bufs=2,
                       name=f"po_{qc}_{sub}_{ch}")
        for i in range(4):
            nc.tensor.matmul(
                po[:], cT[:, i, sub * 128:(sub + 1) * 128],
                wo_sb[:, i, ch * 512:(ch + 1) * 512],
                start=(i == 0), stop=(i == 3))
        if tail and ch % 2 == 0:
            nc.scalar.copy(ost[:, ch * 512:(ch + 1) * 512], po[:])
        else:
            nc.vector.tensor_copy(ost[:, ch * 512:(ch + 1) * 512], po[:])
        rows = slice(qc * 512 + sub * 128, qc * 512 + (sub + 1) * 128)
        if tail:
            cols = slice(ch * 512, (ch + 1) * 512)
            nc.sync.dma_start(out[rows, cols], ost[:, cols])
        elif ch % 2 == 1:
            cols = slice((ch - 1) * 512, (ch + 1) * 512)
            nc.sync.dma_start(out[rows, cols], ost[:, cols])

    prev = None            # (qc, ctxT) pending output projection
    pend = []              # [(qc, hd, expt, ctxT)] ctx not yet enqueued
    # out-proj chunks flow through a global FIFO, drained unevenly across
    # the later qcs (qc3 has no next-qc Q-proj fillers, so it gets more).
    outq = deque()
    osts = {}

    def push_out_chunks(pqc, pcT):
        for sub in range(4):
            for ch in range(4):
                def mk(s=sub, c=ch, t=pcT, q=pqc):
                    if (q, s) not in osts:
                        osts[(q, s)] = ctxp.tile(
                            [128, DM], BF16, tag="ost", bufs=2,
                            name=f"ost_{q}_{s}")
                    out_chunk(q, s, c, t, osts[(q, s)])
                outq.append(mk)

    OUT_PER_HEAD = {
        1: [2, 1, 2, 1, 2, 1, 2, 1],
        2: [2, 1, 2, 1, 2, 1, 2, 1],
        3: [2, 2, 3, 3, 3, 3, 4, 4],
    }

    def warm_fill(n, tag):
        # idle-filler matmuls in Act-starved stretches: keep the PE clock
        # ramped so the next real chain dispatches at the fast rate
        for _ in range(n):
            def wf():
                pw = psum.tile([128, DH + 1], F32, tag="cx", bufs=2,
                               name=f"warmf_{warmup.i}")
                warmup.i += 1
                nc.tensor.matmul(pw[:], warm[:], warm[:, 0:DH + 1])
            fillers.append((110.0, wf, tag))

    for qc in range(4):
        ctxT = ctxp.tile([128, 4, 512], BF16, tag="ctxT", name=f"ctxT_{qc}")
        defer = 3 if qc == 0 else (2 if qc < 3 else 1)
        # stream next q chunk (and wo after the last input chunk)
        q_next = dma_q(qc + 1) if qc < 3 else None
        if qc == 0:
            nc.sync.dma_start(wo_sb[:], wo.rearrange("(i p) d -> p i d", p=128))
        # order barrier: qtp(qc) writes must be emitted before qc's pairs
        if qc > 0:
            drain_tag(f"qp{qc}")
        for hd in range(8):
            if qc == 0 and hd < 4:
                for t in range(4 * hd, 4 * hd + 4):
                    fillers.append((860.0, lambda t=t: v_proj(t), "vp"))
            # at qc start flush ALL pending ctx (so out fillers of the
            # previous qc are enqueued after every ctx write of that qc)
            want = 0 if hd == 0 else defer
            while len(pend) > want:
                pq2, phd, pexp, pcT2 = pend.pop(0)
                for sub in range(4):
                    fillers.append(
                        (440.0, ctx_sub(pq2, phd, sub, pexp, pcT2),
                         f"ctx{pq2}"))
            if hd == 0 and prev is not None:
                push_out_chunks(prev[0], prev[1])
            if qc > 0:
                for _ in range(OUT_PER_HEAD[qc][hd]):
                    if outq:
                        fillers.append((860.0, outq.popleft(), "out"))

            if qc < 3 and hd >= 4:
                fillers.append(
                    (3400.0, lambda f=hd - 4, qn=q_next, qc1=qc + 1: (
                        q_proj_one(qc1, qn, f)), f"qp{qc + 1}"))
            expt = expp.tile([128, NKT, 512], BF16, tag="exp",
                             name=f"exp_{qc}_{hd}")
            for kt2 in range(NKT // 2):
                pair(qc, hd, kt2, expt)
                drain(PAIR_NS)
            pend.append((qc, hd, expt, ctxT))
        prev = (qc, ctxT)

    # tail: remaining ctx heads, fillers, final output projection
    while pend:
        pq2, phd, pexp, pcT2 = pend.pop(0)
        for sub in range(4):
            ctx_sub(pq2, phd, sub, pexp, pcT2)()
            drain(PAIR_NS)
    drain(1e12)
    pqc, pcT = prev
    for sub in range(4):
        ost = ctxp.tile([128, DM], BF16, tag="ost", bufs=2,
                        name=f"ost_{pqc}_{sub}")
        for ch in range(4):
            out_chunk(pqc, sub, ch, pcT, ost, tail=True)


def _build():
    nc = bacc.Bacc("TRN2", target_bir_lowering=False, debug=False, num_devices=NC)
    qT = nc.dram_tensor("qT", [DM, S], BF16, kind="ExternalInput")
    kT = nc.dram_tensor("kT", [DM, S], BF16, kind="ExternalInput")
    vT = nc.dram_tensor("vT", [DM, S], BF16, kind="ExternalInput")
    # weights pre-arranged on host for contiguous-per-partition DMA
    wq = nc.dram_tensor("wq", [128, 4 * DT * 128], BF16, kind="ExternalInput")
    wk = nc.dram_tensor("wk", [128, DT * DKV], BF16, kind="ExternalInput")
    wv = nc.dram_tensor("wv", [128, DT * DKV], BF16, kind="ExternalInput")
    wo = nc.dram_tensor("wo", [DQ, DM], BF16, kind="ExternalInput")
    out = nc.dram_tensor("out", [S, DM], BF16, kind="ExternalOutput")
    with tile.TileContext(nc) as tc:
        with ExitStack() as ctx:
            _emit(ctx, tc, qT.ap(), kT.ap(), vT.ap(), wq.ap(), wk.ap(),
                  wv.ap(), wo.ap(), out.ap())
    nc.compile()
    return nc


def _make_runner(nc, n_cores=NC):
    """Build the sharded jit callable once; reuse across kernel() calls."""
    bass2jax.install_neuronx_cc_hook()
    partition_name = nc.partition_id_tensor.name if nc.partition_id_tensor else None
    in_names, out_names, out_avals, zero_outs = [], [], [], []
    for alloc in nc.m.functions[0].allocations:
        if not isinstance(alloc, mybir.MemoryLocationSet):
            continue
        name = alloc.memorylocations[0].name
        if alloc.kind == "ExternalInput":
            if name != partition_name:
                in_names.append(name)
        elif alloc.kind == "ExternalOutput":
            out_names.append(name)
            shape = tuple(alloc.tensor_shape)
            dtype = mybir.dt.np(alloc.dtype)
            out_avals.append(jax.core.ShapedArray(shape, dtype))
            zero_outs.append(np.zeros(shape, dtype))
    n_params = len(in_names)
    n_outs = len(out_avals)
    in_names_all = in_names + out_names
    if partition_name is not None:
        in_names_all.append(partition_name)
    donate = tuple(range(n_params, n_params + n_outs))

    def _body(*args):
        operands = list(args)
        if partition_name is not None:
            operands.append(bass2jax.partition_id_tensor())
        outs = bass2jax._bass_exec_p.bind(
            *operands,
            out_avals=tuple(out_avals),
            in_names=tuple(in_names_all),
            out_names=tuple(out_names),
            lowering_input_output_aliases=(),
            sim_require_finite=True,
            sim_require_nnan=True,
            nc=nc,
        )
        return tuple(outs)

    devices = jax.devices()[:n_cores]
    mesh = Mesh(np.asarray(devices), ("core",))
    in_specs = (PartitionSpec("core"),) * (n_params + n_outs)
    out_specs = (PartitionSpec("core"),) * len(out_names)
    sharded = jax.jit(
        shard_map(_body, mesh=mesh, in_specs=in_specs, out_specs=out_specs,
                  check_rep=False),
        donate_argnums=donate, keep_unused=True)
    sh = NamedSharding(mesh, PartitionSpec("core"))
    return sharded, in_names, out_names, zero_outs, sh


def _run(in_maps):
    if "nc" not in _cache:
        _cache["nc"] = _build()
    if "runner" not in _cache:
        _cache["runner"] = _make_runner(_cache["nc"])
    sharded, in_names, out_names, zero_outs, sh = _cache["runner"]
    n = NC
    concat_in = [
        jax.device_put(
            np.concatenate([np.asarray(in_maps[c][nm]) for c in range(n)], 0), sh)
        for nm in in_names
    ]
    zeros = [
        jax.device_put(np.zeros((n * z.shape[0], *z.shape[1:]), z.dtype), sh)
        for z in zero_outs
    ]
    outs = sharded(*concat_in, *zeros)
    i = out_names.index("out")
    arr = np.asarray(outs[i])           # [NC*S, DM]
    return arr.reshape(n, S, DM)


def kernel(q, k, v, Wq, Wk, Wv, Wo):
    q = np.asarray(q, dtype=np.float32)
    k = np.asarray(k, dtype=np.float32)
    v = np.asarray(v, dtype=np.float32)
    bf = ml_dtypes.bfloat16
    qTh = np.ascontiguousarray(q.astype(bf).transpose(0, 2, 1))
    kTh = np.ascontiguousarray(k.astype(bf).transpose(0, 2, 1))
    vTh = np.ascontiguousarray(v.astype(bf).transpose(0, 2, 1))
    Wq32 = np.asarray(Wq, dtype=np.float32)
    Wk32 = np.asarray(Wk, dtype=np.float32)
    Wv32 = np.asarray(Wv, dtype=np.float32)
    Wo32 = np.asarray(Wo, dtype=np.float32)

    in_maps = []
    for c in range(NC):
        b, g = c // 4, c % 4
        # head (u, f) -> qtp ftile f, rows u*64:(u+1)*64
        wq_s = Wq32[:, g * DQ:(g + 1) * DQ].reshape(DM, 2, 4, DH)
        wq_p = wq_s.transpose(0, 2, 1, 3).reshape(DM, DQ)
        # device layout [p, f, dt, d]: wq_pre[p, f*DT*128 + dt*128 + x]
        # = wq_p[dt*128 + p, f*128 + x]
        wq_pre = (wq_p.reshape(DT, 128, 4, 128)
                  .transpose(1, 2, 0, 3).reshape(128, 4 * DT * 128))
        # device layout [p, dt, m]: w_pre[p, dt*DKV + m] = w[dt*128 + p, m]
        wk_s = Wk32[:, g * DKV:(g + 1) * DKV]
        wk_pre = wk_s.reshape(DT, 128, DKV).transpose(1, 0, 2).reshape(128, -1)
        wv_s = Wv32[:, g * DKV:(g + 1) * DKV]
        wv_pre = wv_s.reshape(DT, 128, DKV).transpose(1, 0, 2).reshape(128, -1)
        wo_s = Wo32[g * DQ:(g + 1) * DQ, :].reshape(2, 4, DH, DM)
        wo_p = wo_s.transpose(1, 0, 2, 3).reshape(DQ, DM)
        in_maps.append({
            "qT": qTh[b], "kT": kTh[b], "vT": vTh[b],
            "wq": np.ascontiguousarray(wq_pre.astype(bf)),
            "wk": np.ascontiguousarray(wk_pre.astype(bf)),
            "wv": np.ascontiguousarray(wv_pre.astype(bf)),
            "wo": np.ascontiguousarray(wo_p.astype(bf)),
        })
    partials = _run(in_maps)            # [8, S, DM] bf16
    out = partials.astype(np.float32, copy=False).reshape(2, 4, S, DM).sum(axis=1)
    return out


# revision 5
# speedup vs baseline: 1.0004x; 1.0004x over previous
"""GQA multi-head attention (B=2, S=2048, D=2048, 32 q-heads / 8 kv-heads)
on 8 Trainium2 NeuronCores.  511.6us baseline -> 332.5us.

Sharding: batch (2-way) x kv-head-pair (4-way). Core c owns batch c//4 and
kv heads {2g, 2g+1} with g = c%4, i.e. q heads 8g..8g+7 (512 q-proj feats).
Wq/Wk/Wv column-sharded, Wo row-sharded; host sums the 4 partials per batch.

Per-core dataflow (bf16 operands, fp32 PSUM; every GEMM at the tensor-engine
cost floor = output-free-size cycles per 128-partition tile):
  K^T [128f, S]  = Wk_c^T @ k^T          (2 kv heads fill all 128 partitions)
  V   [S, 128f]  via lhsT=vT chunk, rhs=Wv_c (tokens on partitions; +ones
                   cols at 64/130 for the softmax denominators)
  Q^T [128, 4, S]: ftile f rows u*64:(u+1)*64 = head (u,f) (u = kv half)
  S^T [kv, q] = ktd[u-half].T-slice @ qtp[u-half, f]   (64-contraction)
  expS = exp(S^T * 1/8) on ACT -> bf16 SBUF  (ACT is the co-critical engine)
  ctx [q, 65] = expS^T-chunk.T @ V_aug   (ones col -> rowsum at col 64)
  cn[128,128] = ctx[:,0:64] * recip(ctx[:,64]) for both kv halves, then one
  XBAR DMA transpose -> ctxT; out [q, dm] = ctxT.T @ Wo_c -> SBUF -> HBM

Schedule: score-pair emissions are the clock; a FIFO of micro-fillers (ctx
chains, DMA transposes, out-proj chunks, quarter Q-proj chains, V-proj
chains) drains ~700ns between pairs. Emission order IS program order in the
Tile framework (a read emitted before a write sees stale data), so fillers
that must precede a consumer are flushed via tagged order barriers. Warm-up
matmuls keep the PE p-state ramped across DMA waits (costs are locked at
dispatch at the current clock); DMA-gated V-proj chains are priority-demoted
so they cannot preempt the score pairs that feed ACT.
"""

from contextlib import ExitStack

import numpy as np
import ml_dtypes

import jax

try:
    jax.config.update("jax_compilation_cache_dir", "/tmp/jax_bass_cache")
    jax.config.update("jax_persistent_cache_min_compile_time_secs", 1.0)
except Exception:
    pass

from jax.sharding import Mesh, PartitionSpec, NamedSharding
from jax.experimental.shard_map import shard_map

import concourse.bass as bass
import concourse.mybir as mybir
import concourse.tile as tile
from concourse import bacc, bass2jax

BF16 = mybir.dt.bfloat16
F32 = mybir.dt.float32
AF = mybir.ActivationFunctionType

B, S, DM = 2, 2048, 2048
NC = 8
DT = DM // 128         # 16 contraction tiles
DH = 64
DQ = 512               # per-core q-proj width (8 heads)
DKV = 128              # per-core kv width (2 heads)
NKT = S // 128         # 16 kv tiles
SCALE = 1.0 / 8.0      # 1/sqrt(64)

_cache = {}


def _emit(ctx, tc, qT, kT, vT, wq, wk, wv, wo, out):
    nc = tc.nc

    pp = ctx.enter_context(tc.tile_pool(name="persist", bufs=1))
    wq_sb = pp.tile([128, 4, DT, 128], BF16, tag="wq")   # [p, f, dt, d]
    wk_sb = pp.tile([128, DT, DKV], BF16, tag="wk")
    wv_sb = pp.tile([128, DT, DKV], BF16, tag="wv")
    wo_sb = pp.tile([128, 4, DM], BF16, tag="wo")
    qtp = pp.tile([128, 4, S], BF16, tag="qtp")      # [u*64+d, f, q]
    ktd = pp.tile([128, S], BF16, tag="ktd")         # [u*64+d, kv]
    vsb = pp.tile([128, NKT, 132], BF16, tag="vsb")  # [kv%128, kvtile, 2x66]
    ident = pp.tile([128, 128], BF16, tag="ident")

    from concourse.masks import make_identity
    make_identity(nc, ident[:])
    nc.gpsimd.memset(vsb[:, :, DH], 1.0)        # ones col, kv head 0
    nc.gpsimd.memset(vsb[:, :, 66 + DH], 1.0)   # ones col, kv head 1
    warm = pp.tile([128, 128], BF16, tag="warm")  # also feeds warm_fill

    stage = ctx.enter_context(tc.tile_pool(name="stage", bufs=3))
    expp = ctx.enter_context(tc.tile_pool(name="expp", bufs=4))
    smal = ctx.enter_context(tc.tile_pool(name="small", bufs=3))
    ctxp = ctx.enter_context(tc.tile_pool(name="ctxp", bufs=2))
    psum = ctx.enter_context(tc.tile_pool(name="psum", bufs=1, space="PSUM"))

    # ---------------- staged input DMA helpers ----------------
    def dma_k(c):
        k_ch = stage.tile([128, DT, 512], BF16, tag="instage", name=f"k_ch_{c}")
        nc.sync.dma_start(
            k_ch[:],
            kT.rearrange("(dt p) s -> p dt s", p=128)[:, :, c * 512:(c + 1) * 512])
        return k_ch

    def dma_q(qc):
        q_ch = stage.tile([128, DT, 512], BF16, tag="instage", name=f"q_ch_{qc}")
        nc.sync.dma_start(
            q_ch[:],
            qT.rearrange("(dt p) s -> p dt s", p=128)[:, :, qc * 512:(qc + 1) * 512])
        return q_ch

    def dma_v(c):
        v_ch = stage.tile([128, DT, 512], BF16, tag="instage", name=f"v_ch_{c}")
        nc.sync.dma_start(
            v_ch[:],
            vT.rearrange("(dt p) s -> p dt s", p=128)[:, :, c * 512:(c + 1) * 512])
        return v_ch

    # Weights arrive pre-arranged on the host so every DMA has >=512B
    # contiguous runs (full DMA rate). K streams in 5 chunks, narrow first,
    # so K-proj and the first score pairs start as early as possible.
    K_CH = [(0, 256), (256, 256), (512, 512), (1024, 512), (1536, 512)]

    def dma_kc(i):
        off, w = K_CH[i]
        k_ch = stage.tile([128, DT, w], BF16, tag="instage", name=f"k_ch_{i}")
        nc.sync.dma_start(
            k_ch[:],
            kT.rearrange("(dt p) s -> p dt s", p=128)[:, :, off:off + w])
        return k_ch

    nc.sync.dma_start(wk_sb[:], wk.rearrange("p (dt m) -> p dt m", m=DKV))
    k_chs = [dma_kc(0)]
    wqr = wq.rearrange("p (f dt m) -> p f dt m", f=4, m=128)
    nc.sync.dma_start(wq_sb[:, 0], wqr[:, 0])
    q_ch0 = dma_q(0)
    k_chs.append(dma_kc(1))
    nc.sync.dma_start(wq_sb[:, 1:4], wqr[:, 1:4])
    nc.sync.dma_start(wv_sb[:], wv.rearrange("p (dt m) -> p dt m", m=DKV))
    k_chs += [dma_kc(2), dma_kc(3), dma_kc(4)]
    v_chs = [dma_v(c) for c in range(4)]

    def k_proj(c):
        off, w = K_CH[c]
        pk = psum.tile([128, w], F32, tag="mm", bufs=2, name=f"pk_{c}")
        for dt in range(DT):
            nc.tensor.matmul(pk[:], wk_sb[:, dt, :], k_chs[c][:, dt, :],
                             start=(dt == 0), stop=(dt == DT - 1))
        nc.vector.tensor_copy(ktd[:, off:off + w], pk[:])

    def v_proj(t):
        # demoted priority: these chains unblock when their vT chunk lands
        # mid-attention; without the demotion they preempt the score pairs
        # that feed ACT (priority inversion -> ACT starvation)
        with tc.high_priority(-400):
            pv = psum.tile([128, DKV], F32, tag="mm", bufs=2, name=f"pv_{t}")
            for dt in range(DT):
                nc.tensor.matmul(
                    pv[:], v_chs[t // 4][:, dt, (t % 4) * 128:(t % 4 + 1) * 128],
                    wv_sb[:, dt, :], start=(dt == 0), stop=(dt == DT - 1))
            # [:, 0:64] -> cols 0:64 (head 0), [:, 64:128] -> cols 66:130
            dst = vsb[:, t, :].rearrange("p (a b) -> p a b", b=66)[:, :, 0:DH]
            nc.vector.tensor_copy(dst, pv[:].rearrange("p (a b) -> p a b", b=DH))

    def q_chain_block(pq, q_ch, f, quarter):
        """4 of the 16 accumulation steps of Q-proj chain f."""
        for dt in range(quarter * 4, quarter * 4 + 4):
            nc.tensor.matmul(
                pq[:], wq_sb[:, f, dt, :], q_ch[:, dt, :],
                start=(dt == 0), stop=(dt == DT - 1))

    from contextlib import contextmanager

    @contextmanager
    def _null():
        yield

    def q_proj_one(qc, q_ch, f, demote=0):
        with tc.high_priority(-demote) if demote else _null():
            pq = psum.tile([128, 512], F32, tag="mm", bufs=2,
                           name=f"pq_{qc}_{f}")
            for quarter in range(4):
                q_chain_block(pq, q_ch, f, quarter)
            nc.vector.tensor_copy(qtp[:, f, qc * 512:(qc + 1) * 512], pq[:])

    # Warm-up matmuls: keep the PE busy (and its p-state ramping toward full
    # clock) while the first input DMAs stream in. Results are never read.
    nc.vector.memset(warm[:], 0.0)

    def warmup(n):
        # Keep the PE continuously busy (matmul p-state stays ramped) while
        # input DMAs stream in; results are never read. Priority order lets
        # real work preempt the remaining warmups the moment it is ready.
        for w in range(n):
            pw = psum.tile([128, 2, 128], F32, tag="sc", bufs=2,
                           name=f"warm_{warmup.i}")
            warmup.i += 1
            nc.tensor.matmul(pw[:, 0], warm[:], warm[:])
            nc.tensor.matmul(pw[:, 1], warm[:], warm[:])
    warmup.i = 0

    # PE preamble: narrow K chunks and Q proj(0) interleaved with arrival
    # order; V proj chains are deferred into the qc0 filler stream. Warmups
    # interleave so the PE never idles (a PE idle gap resets the clock ramp
    # and the next chain would dispatch at the cold rate).
    warmup(25)
    k_proj(0)
    warmup(34)
    q_proj_one(0, q_ch0, 0)
    k_proj(1)
    for f in range(1, 4):
        q_proj_one(0, q_ch0, f)
    for c in range(2, 5):
        k_proj(c)

    # ---------------- pipelined attention + output projection ----------------
    # Score-pair emissions are the clock; between pairs we drain ~PAIR_NS of
    # deferred PE micro-tasks (ctx chains, transposes, out-proj chunks,
    # quarter Q-proj chains, V-proj chains) from a FIFO.
    from collections import deque
    fillers = deque()   # (cost_ns, thunk, tag)
    PAIR_NS = 700.0

    def drain(budget):
        got = 0.0
        while fillers and got < budget:
            cost, thunk, _ = fillers.popleft()
            thunk()
            got += cost

    def drain_tag(tag):
        """Emit (FIFO) until no filler with `tag` remains — order barrier."""
        while any(t == tag for _, _, t in fillers):
            cost, thunk, _ = fillers.popleft()
            thunk()

    def pair(qc, hd, kt2, expt):
        u, f = hd // 4, hd % 4
        lo, hi = u * DH, (u + 1) * DH
        ps = psum.tile([128, 2, 512], F32, tag="sc", bufs=2,
                       name=f"ps_{qc}_{hd}_{kt2}")
        for j in range(2):
            kt = 2 * kt2 + j
            nc.tensor.matmul(
                ps[:, j, :],
                ktd[lo:hi, kt * 128:(kt + 1) * 128],
                qtp[lo:hi, f, qc * 512:(qc + 1) * 512])
        nc.scalar.activation(
            expt[:, 2 * kt2:2 * kt2 + 2, :], ps[:], AF.Exp, scale=SCALE)

    # cn tiles buffer the normalized ctx of BOTH kv halves of an ftile
    # ([128 q, 128] = head f in cols 0:64, head f+4 in 64:128) so a single
    # XBAR DMA transpose produces the full-partition ctxT block — no PE
    # transpose, no DVE psum->sbuf copy.
    cns = {}

    def ctx_sub(qc, hd, sub, expt, cT):
        u, f = hd // 4, hd % 4

        def chain():
            pc = psum.tile([128, DH + 1], F32, tag="cx", bufs=2,
                           name=f"pc_{qc}_{hd}_{sub}")
            for kt in range(NKT):
                nc.tensor.matmul(
                    pc[:], expt[:, kt, sub * 128:(sub + 1) * 128],
                    vsb[:, kt, u * 66:u * 66 + DH + 1],
                    start=(kt == 0), stop=(kt == NKT - 1))
            rc = smal.tile([128, 1], F32, tag="rc", name=f"rc_{qc}_{hd}_{sub}")
            nc.vector.reciprocal(rc[:], pc[:, DH:DH + 1])
            if (qc, f, sub) not in cns:
                cns[(qc, f, sub)] = smal.tile(
                    [128, 128], BF16, tag="cn", bufs=20,
                    name=f"cn_{qc}_{f}_{sub}")
            cn = cns[(qc, f, sub)]
            nc.vector.tensor_scalar_mul(
                cn[:, u * DH:(u + 1) * DH], pc[:, 0:DH], rc[:])
            if u == 1:
                del cns[(qc, f, sub)]

                def transp():
                    nc.sync.dma_start_transpose(
                        cT[:, f, sub * 128:(sub + 1) * 128], cn[:])

                # near the FRONT (slightly deferred): must stay ahead of any
                # out-proj filler that reads this ctxT subtile.
                fillers.insert(min(2, len(fillers)),
                               (60.0, transp, f"ctx{qc}"))

        return chain

    def out_chunk(qc, sub, ch, cT, ost, tail=False):
        po = psum.tile([128, 512], F32, tag="mm", bufs=2,
                       name=f"po_{qc}_{sub}_{ch}")
        for i in range(4):
            nc.tensor.matmul(
                po[:], cT[:, i, sub * 128:(sub + 1) * 128],
                wo_sb[:, i, ch * 512:(ch + 1) * 512],
                start=(i == 0), stop=(i == 3))
        if tail and ch % 2 == 0:
            nc.scalar.copy(ost[:, ch * 512:(ch + 1) * 512], po[:])
        else:
            nc.vector.tensor_copy(ost[:, ch * 512:(ch + 1) * 512], po[:])
        rows = slice(qc * 512 + sub * 128, qc * 512 + (sub + 1) * 128)
        if tail:
            cols = slice(ch * 512, (ch + 1) * 512)
            nc.sync.dma_start(out[rows, cols], ost[:, cols])
        elif ch % 2 == 1:
            cols = slice((ch - 1) * 512, (ch + 1) * 512)
            nc.sync.dma_start(out[rows, cols], ost[:, cols])

    prev = None            # (qc, ctxT) pending output projection
    pend = []              # [(qc, hd, expt, ctxT)] ctx not yet enqueued
    # out-proj chunks flow through a global FIFO, drained unevenly across
    # the later qcs (qc3 has no next-qc Q-proj fillers, so it gets more).
    outq = deque()
    osts = {}

    def push_out_chunks(pqc, pcT):
        for sub in range(4):
            for ch in range(4):
                def mk(s=sub, c=ch, t=pcT, q=pqc):
                    if (q, s) not in osts:
                        osts[(q, s)] = ctxp.tile(
                            [128, DM], BF16, tag="ost", bufs=2,
                            name=f"ost_{q}_{s}")
                    out_chunk(q, s, c, t, osts[(q, s)])
                outq.append(mk)

    OUT_PER_HEAD = {
        1: [2, 1, 2, 1, 2, 1, 2, 1],
        2: [2, 1, 2, 1, 2, 1, 2, 1],
        3: [2, 2, 3, 3, 3, 3, 4, 4],
    }

    def warm_fill(n, tag):
        # idle-filler matmuls in Act-starved stretches: keep the PE clock
        # ramped so the next real chain dispatches at the fast rate
        for _ in range(n):
            def wf():
                pw = psum.tile([128, DH + 1], F32, tag="cx", bufs=2,
                               name=f"warmf_{warmup.i}")
                warmup.i += 1
                nc.tensor.matmul(pw[:], warm[:], warm[:, 0:DH + 1])
            fillers.append((110.0, wf, tag))

    for qc in range(4):
        ctxT = ctxp.tile([128, 4, 512], BF16, tag="ctxT", name=f"ctxT_{qc}")
        defer = 3 if qc == 0 else (2 if qc < 3 else 1)
        # stream next q chunk (and wo after the last input chunk)
        q_next = dma_q(qc + 1) if qc < 3 else None
        if qc == 0:
            nc.sync.dma_start(wo_sb[:], wo.rearrange("(i p) d -> p i d", p=128))
        # order barrier: qtp(qc) writes must be emitted before qc's pairs
        if qc > 0:
            drain_tag(f"qp{qc}")
        for hd in range(8):
            if qc == 0 and hd < 4:
                for t in range(4 * hd, 4 * hd + 4):
                    fillers.append((860.0, lambda t=t: v_proj(t), "vp"))
            # at qc start flush ALL pending ctx (so out fillers of the
            # previous qc are enqueued after every ctx write of that qc)
            want = 0 if hd == 0 else defer
            while len(pend) > want:
                pq2, phd, pexp, pcT2 = pend.pop(0)
                for sub in range(4):
                    fillers.append(
                        (440.0, ctx_sub(pq2, phd, sub, pexp, pcT2),
                         f"ctx{pq2}"))
            if hd == 0 and prev is not None:
                push_out_chunks(prev[0], prev[1])
            if qc > 0:
                for _ in range(OUT_PER_HEAD[qc][hd]):
                    if outq:
                        fillers.append((860.0, outq.popleft(), "out"))

            if qc < 3 and hd >= 4:
                fillers.append(
                    (3400.0, lambda f=hd - 4, qn=q_next, qc1=qc + 1: (
                        q_proj_one(qc1, qn, f)), f"qp{qc + 1}"))
            expt = expp.tile([128, NKT, 512], BF16, tag="exp",
                             name=f"exp_{qc}_{hd}")
            for kt2 in range(NKT // 2):
                pair(qc, hd, kt2, expt)
                drain(PAIR_NS)
            pend.append((qc, hd, expt, ctxT))
        prev = (qc, ctxT)

    # tail: remaining ctx heads, fillers, final output projection
    while pend:
        pq2, phd, pexp, pcT2 = pend.pop(0)
        for sub in range(4):
            ctx_sub(pq2, phd, sub, pexp, pcT2)()
            drain(PAIR_NS)
    drain(1e12)
    pqc, pcT = prev
    for sub in range(4):
        ost = ctxp.tile([128, DM], BF16, tag="ost", bufs=2,
                        name=f"ost_{pqc}_{sub}")
        for ch in range(4):
            out_chunk(pqc, sub, ch, pcT, ost, tail=True)


def _build():
    nc = bacc.Bacc("TRN2", target_bir_lowering=False, debug=False, num_devices=NC)
    qT = nc.dram_tensor("qT", [DM, S], BF16, kind="ExternalInput")
    kT = nc.dram_tensor("kT", [DM, S], BF16, kind="ExternalInput")
    vT = nc.dram_tensor("vT", [DM, S], BF16, kind="ExternalInput")
    # weights pre-arranged on host for contiguous-per-partition DMA
    wq = nc.dram_tensor("wq", [128, 4 * DT * 128], BF16, kind="ExternalInput")
    wk = nc.dram_tensor("wk", [128, DT * DKV], BF16, kind="ExternalInput")
    wv = nc.dram_tensor("wv", [128, DT * DKV], BF16, kind="ExternalInput")
    wo = nc.dram_tensor("wo", [DQ, DM], BF16, kind="ExternalInput")
    out = nc.dram_tensor("out", [S, DM], BF16, kind="ExternalOutput")
    with tile.TileContext(nc) as tc:
        with ExitStack() as ctx:
            _emit(ctx, tc, qT.ap(), kT.ap(), vT.ap(), wq.ap(), wk.ap(),
                  wv.ap(), wo.ap(), out.ap())
    nc.compile()
    return nc


def _make_runner(nc, n_cores=NC):
    """Build the sharded jit callable once; reuse across kernel() calls."""
    bass2jax.install_neuronx_cc_hook()
    partition_name = nc.partition_id_tensor.name if nc.partition_id_tensor else None
    in_names, out_names, out_avals, zero_outs = [], [], [], []
    for alloc in nc.m.functions[0].allocations:
        if not isinstance(alloc, mybir.MemoryLocationSet):
            continue
        name = alloc.memorylocations[0].name
        if alloc.kind == "ExternalInput":
            if name != partition_name:
                in_names.append(name)
        elif alloc.kind == "ExternalOutput":
            out_names.append(name)
            shape = tuple(alloc.tensor_shape)
            dtype = mybir.dt.np(alloc.dtype)
            out_avals.append(jax.core.ShapedArray(shape, dtype))
            zero_outs.append(np.zeros(shape, dtype))
    n_params = len(in_names)
    n_outs = len(out_avals)
    in_names_all = in_names + out_names
    if partition_name is not None:
        in_names_all.append(partition_name)
    donate = tuple(range(n_params, n_params + n_outs))

    def _body(*args):
        operands = list(args)
        if partition_name is not None:
            operands.append(bass2jax.partition_id_tensor())
        outs = bass2jax._bass_exec_p.bind(
            *operands,
            out_avals=tuple(out_avals),
            in_names=tuple(in_names_all),
            out_names=tuple(out_names),
            lowering_input_output_aliases=(),
            sim_require_finite=True,
            sim_require_nnan=True,
            nc=nc,
        )
        return tuple(outs)

    devices = jax.devices()[:n_cores]
    mesh = Mesh(np.asarray(devices), ("core",))
    in_specs = (PartitionSpec("core"),) * (n_params + n_outs)
    out_specs = (PartitionSpec("core"),) * len(out_names)
    sharded = jax.jit(
        shard_map(_body, mesh=mesh, in_specs=in_specs, out_specs=out_specs,
                  check_rep=False),
        donate_argnums=donate, keep_unused=True)
    sh = NamedSharding(mesh, PartitionSpec("core"))
    return sharded, in_names, out_names, zero_outs, sh


def _run(in_maps):
    if "nc" not in _cache:
        _cache["nc"] = _build()
    if "runner" not in _cache:
        _cache["runner"] = _make_runner(_cache["nc"])
    sharded, in_names, out_names, zero_outs, sh = _cache["runner"]
    n = NC
    concat_in = [
        jax.device_put(
            np.concatenate([np.asarray(in_maps[c][nm]) for c in range(n)], 0), sh)
        for nm in in_names
    ]
    zeros = [
        jax.device_put(np.zeros((n * z.shape[0], *z.shape[1:]), z.dtype), sh)
        for z in zero_outs
    ]
    outs = sharded(*concat_in, *zeros)
    i = out_names.index("out")
    arr = np.asarray(outs[i])           # [NC*S, DM]
    return arr.reshape(n, S, DM)


def kernel(q, k, v, Wq, Wk, Wv, Wo):
    q = np.asarray(q, dtype=np.float32)
    k = np.asarray(k, dtype=np.float32)
    v = np.asarray(v, dtype=np.float32)
    bf = ml_dtypes.bfloat16
    qTh = np.ascontiguousarray(q.astype(bf).transpose(0, 2, 1))
    kTh = np.ascontiguousarray(k.astype(bf).transpose(0, 2, 1))
    vTh = np.ascontiguousarray(v.astype(bf).transpose(0, 2, 1))
    Wq32 = np.asarray(Wq, dtype=np.float32)
    Wk32 = np.asarray(Wk, dtype=np.float32)
    Wv32 = np.asarray(Wv, dtype=np.float32)
    Wo32 = np.asarray(Wo, dtype=np.float32)

    in_maps = []
    for c in range(NC):
        b, g = c // 4, c % 4
        # head (u, f) -> qtp ftile f, rows u*64:(u+1)*64
        wq_s = Wq32[:, g * DQ:(g + 1) * DQ].reshape(DM, 2, 4, DH)
        wq_p = wq_s.transpose(0, 2, 1, 3).reshape(DM, DQ)
        # device layout [p, f, dt, d]: wq_pre[p, f*DT*128 + dt*128 + x]
        # = wq_p[dt*128 + p, f*128 + x]
        wq_pre = (wq_p.reshape(DT, 128, 4, 128)
                  .transpose(1, 2, 0, 3).reshape(128, 4 * DT * 128))
        # device layout [p, dt, m]: w_pre[p, dt*DKV + m] = w[dt*128 + p, m]
        wk_s = Wk32[:, g * DKV:(g + 1) * DKV]
        wk_pre = wk_s.reshape(DT, 128, DKV).transpose(1, 0, 2).reshape(128, -1)
        wv_s = Wv32[:, g * DKV:(g + 1) * DKV]
        wv_pre = wv_s.reshape(DT, 128, DKV).transpose(1, 0, 2).reshape(128, -1)
        wo_s = Wo32[g * DQ:(g + 1) * DQ, :].reshape(2, 4, DH, DM)
        wo_p = wo_s.transpose(1, 0, 2, 3).reshape(DQ, DM)
        in_maps.append({
            "qT": qTh[b], "kT": kTh[b], "vT": vTh[b],
            "wq": np.ascontiguousarray(wq_pre.astype(bf)),
            "wk": np.ascontiguousarray(wk_pre.astype(bf)),
            "wv": np.ascontiguousarray(wv_pre.astype(bf)),
            "wo": np.ascontiguousarray(wo_p.astype(bf)),
        })
    partials = _run(in_maps)            # [8, S, DM] bf16
    out = partials.astype(np.float32, copy=False).reshape(2, 4, S, DM).sum(axis=1)
    return out


# revision 6
# speedup vs baseline: 1.0100x; 1.0096x over previous
"""GQA multi-head attention (B=2, S=2048, D=2048, 32 q-heads / 8 kv-heads)
on 8 Trainium2 NeuronCores.

Sharding: batch (2-way) x kv-head-pair (4-way). Core c owns batch c//4 and
kv heads {2g, 2g+1} with g = c%4, i.e. q heads 8g..8g+7 (512 q-proj feats).
Wq/Wk/Wv column-sharded, Wo row-sharded; host sums the 4 partials per batch.

Per-core dataflow (bf16 operands, fp32 PSUM):
  K^T [128f, S]  = Wk_c^T @ k^T          (2 kv heads fill all 128 partitions)
  V   [S, 128f]  = v-tiles^T... via lhsT=vT chunk, rhs=Wv_c (tokens on
                   partitions; +ones cols for softmax denominators)
  Q^T [128, 4, S]: ftile f rows u*64:(u+1)*64 = head (u,f) (u = kv half)
  S^T [kv, q] = ktd[u-half].T-slice @ qtp[u-half, f]   (64-contraction)
  expS = exp(S^T * 1/8) on ACT -> bf16 SBUF
  ctx [q, 65] = expS^T-chunk.T @ V_aug   (ones col -> rowsum at col 64)
  cnorm = ctx[:,0:64] * recip(ctx[:,64]) (DVE per-partition scalar mul)
  ctxT via PE transpose; out [q, dm] = ctxT.T @ Wo_c -> DMA fp32 psum->HBM
"""

from contextlib import ExitStack

import numpy as np
import ml_dtypes

import jax

try:
    jax.config.update("jax_compilation_cache_dir", "/tmp/jax_bass_cache")
    jax.config.update("jax_persistent_cache_min_compile_time_secs", 1.0)
except Exception:
    pass

from jax.sharding import Mesh, PartitionSpec, NamedSharding
from jax.experimental.shard_map import shard_map

import concourse.bass as bass
import concourse.mybir as mybir
import concourse.tile as tile
from concourse import bacc, bass2jax

BF16 = mybir.dt.bfloat16
F32 = mybir.dt.float32
AF = mybir.ActivationFunctionType

B, S, DM = 2, 2048, 2048
NC = 8
DT = DM // 128         # 16 contraction tiles
DH = 64
DQ = 512               # per-core q-proj width (8 heads)
DKV = 128              # per-core kv width (2 heads)
NKT = S // 128         # 16 kv tiles
SCALE = 1.0 / 8.0      # 1/sqrt(64)

_cache = {}


def _emit(ctx, tc, qT, kT, vT, wq, wk, wv, wo, out):
    nc = tc.nc

    pp = ctx.enter_context(tc.tile_pool(name="persist", bufs=1))
    wq_sb = pp.tile([128, 4, DT, 128], BF16, tag="wq")   # [p, f, dt, d]
    wk_sb = pp.tile([128, DT, DKV], BF16, tag="wk")
    wv_sb = pp.tile([128, DT, DKV], BF16, tag="wv")
    wo_sb = pp.tile([128, 4, DM], BF16, tag="wo")
    qtp = pp.tile([128, 4, S], BF16, tag="qtp")      # [u*64+d, f, q]
    ktd = pp.tile([128, S], BF16, tag="ktd")         # [u*64+d, kv]
    vsb = pp.tile([128, NKT, 132], BF16, tag="vsb")  # [kv%128, kvtile, 2x66]
    ident = pp.tile([128, 128], BF16, tag="ident")

    from concourse.masks import make_identity
    make_identity(nc, ident[:])
    nc.gpsimd.memset(vsb[:, :, DH], 1.0)        # ones col, kv head 0
    nc.gpsimd.memset(vsb[:, :, 66 + DH], 1.0)   # ones col, kv head 1
    warm = pp.tile([128, 128], BF16, tag="warm")  # also feeds warm_fill

    stage = ctx.enter_context(tc.tile_pool(name="stage", bufs=3))
    expp = ctx.enter_context(tc.tile_pool(name="expp", bufs=4))
    smal = ctx.enter_context(tc.tile_pool(name="small", bufs=3))
    ctxp = ctx.enter_context(tc.tile_pool(name="ctxp", bufs=2))
    psum = ctx.enter_context(tc.tile_pool(name="psum", bufs=1, space="PSUM"))

    # ---------------- staged input DMA helpers ----------------
    def dma_k(c):
        k_ch = stage.tile([128, DT, 512], BF16, tag="instage", name=f"k_ch_{c}")
        nc.sync.dma_start(
            k_ch[:],
            kT.rearrange("(dt p) s -> p dt s", p=128)[:, :, c * 512:(c + 1) * 512])
        return k_ch

    def dma_q(qc):
        q_ch = stage.tile([128, DT, 512], BF16, tag="instage", name=f"q_ch_{qc}")
        nc.sync.dma_start(
            q_ch[:],
            qT.rearrange("(dt p) s -> p dt s", p=128)[:, :, qc * 512:(qc + 1) * 512])
        return q_ch

    def dma_v(c):
        v_ch = stage.tile([128, DT, 512], BF16, tag="instage", name=f"v_ch_{c}")
        nc.sync.dma_start(
            v_ch[:],
            vT.rearrange("(dt p) s -> p dt s", p=128)[:, :, c * 512:(c + 1) * 512])
        return v_ch

    # Weights arrive pre-arranged on the host so every DMA has >=512B
    # contiguous runs (full DMA rate). K streams in 5 chunks, narrow first,
    # so K-proj and the first score pairs start as early as possible.
    K_CH = [(0, 256), (256, 256), (512, 512), (1024, 512), (1536, 512)]

    def dma_kc(i):
        off, w = K_CH[i]
        k_ch = stage.tile([128, DT, w], BF16, tag="instage", name=f"k_ch_{i}")
        nc.sync.dma_start(
            k_ch[:],
            kT.rearrange("(dt p) s -> p dt s", p=128)[:, :, off:off + w])
        return k_ch

    nc.sync.dma_start(wk_sb[:], wk.rearrange("p (dt m) -> p dt m", m=DKV))
    k_chs = [dma_kc(0)]
    wqr = wq.rearrange("p (f dt m) -> p f dt m", f=4, m=128)
    nc.sync.dma_start(wq_sb[:, 0], wqr[:, 0])
    q_ch0 = dma_q(0)
    k_chs.append(dma_kc(1))
    nc.sync.dma_start(wq_sb[:, 1:4], wqr[:, 1:4])
    nc.sync.dma_start(wv_sb[:], wv.rearrange("p (dt m) -> p dt m", m=DKV))
    k_chs += [dma_kc(2), dma_kc(3), dma_kc(4)]
    v_chs = [dma_v(c) for c in range(4)]

    def k_proj(c):
        off, w = K_CH[c]
        pk = psum.tile([128, w], F32, tag="mm", bufs=2, name=f"pk_{c}")
        for dt in range(DT):
            nc.tensor.matmul(pk[:], wk_sb[:, dt, :], k_chs[c][:, dt, :],
                             start=(dt == 0), stop=(dt == DT - 1))
        nc.vector.tensor_copy(ktd[:, off:off + w], pk[:])

    def v_proj(t):
        # demoted priority: these chains unblock when their vT chunk lands
        # mid-attention; without the demotion they preempt the score pairs
        # that feed ACT (priority inversion -> ACT starvation)
        with tc.high_priority(-400):
            pv = psum.tile([128, DKV], F32, tag="mm", bufs=2, name=f"pv_{t}")
            for dt in range(DT):
                nc.tensor.matmul(
                    pv[:], v_chs[t // 4][:, dt, (t % 4) * 128:(t % 4 + 1) * 128],
                    wv_sb[:, dt, :], start=(dt == 0), stop=(dt == DT - 1))
            # [:, 0:64] -> cols 0:64 (head 0), [:, 64:128] -> cols 66:130
            dst = vsb[:, t, :].rearrange("p (a b) -> p a b", b=66)[:, :, 0:DH]
            nc.vector.tensor_copy(dst, pv[:].rearrange("p (a b) -> p a b", b=DH))

    def q_chain_block(pq, q_ch, f, quarter):
        """4 of the 16 accumulation steps of Q-proj chain f."""
        for dt in range(quarter * 4, quarter * 4 + 4):
            nc.tensor.matmul(
                pq[:], wq_sb[:, f, dt, :], q_ch[:, dt, :],
                start=(dt == 0), stop=(dt == DT - 1))

    from contextlib import contextmanager

    @contextmanager
    def _null():
        yield

    def q_proj_one(qc, q_ch, f, demote=0):
        with tc.high_priority(-demote) if demote else _null():
            pq = psum.tile([128, 512], F32, tag="mm", bufs=2,
                           name=f"pq_{qc}_{f}")
            for quarter in range(4):
                q_chain_block(pq, q_ch, f, quarter)
            nc.vector.tensor_copy(qtp[:, f, qc * 512:(qc + 1) * 512], pq[:])

    # Warm-up matmuls: keep the PE busy (and its p-state ramping toward full
    # clock) while the first input DMAs stream in. Results are never read.
    nc.vector.memset(warm[:], 0.0)

    def warmup(n):
        # Keep the PE continuously busy (matmul p-state stays ramped) while
        # input DMAs stream in; results are never read. Priority order lets
        # real work preempt the remaining warmups the moment it is ready.
        for w in range(n):
            pw = psum.tile([128, 2, 128], F32, tag="sc", bufs=2,
                           name=f"warm_{warmup.i}")
            warmup.i += 1
            nc.tensor.matmul(pw[:, 0], warm[:], warm[:])
            nc.tensor.matmul(pw[:, 1], warm[:], warm[:])
    warmup.i = 0

    # PE preamble: narrow K chunks and Q proj(0) interleaved with arrival
    # order; V proj chains are deferred into the qc0 filler stream. Warmups
    # interleave so the PE never idles (a PE idle gap resets the clock ramp
    # and the next chain would dispatch at the cold rate).
    warmup(25)
    k_proj(0)
    warmup(34)
    q_proj_one(0, q_ch0, 0)
    k_proj(1)
    for f in range(1, 4):
        q_proj_one(0, q_ch0, f)
    for c in range(2, 5):
        k_proj(c)

    # ---------------- pipelined attention + output projection ----------------
    # Score-pair emissions are the clock; between pairs we drain ~PAIR_NS of
    # deferred PE micro-tasks (ctx chains, transposes, out-proj chunks,
    # quarter Q-proj chains, V-proj chains) from a FIFO.
    from collections import deque
    fillers = deque()   # (cost_ns, thunk, tag)
    PAIR_NS = 700.0

    def drain(budget):
        got = 0.0
        while fillers and got < budget:
            cost, thunk, _ = fillers.popleft()
            thunk()
            got += cost

    def drain_tag(tag):
        """Emit (FIFO) until no filler with `tag` remains — order barrier."""
        while any(t == tag for _, _, t in fillers):
            cost, thunk, _ = fillers.popleft()
            thunk()

    def pair(qc, hd, kt2, expt):
        u, f = hd // 4, hd % 4
        lo, hi = u * DH, (u + 1) * DH
        ps = psum.tile([128, 2, 512], F32, tag="sc", bufs=2,
                       name=f"ps_{qc}_{hd}_{kt2}")
        for j in range(2):
            kt = 2 * kt2 + j
            nc.tensor.matmul(
                ps[:, j, :],
                ktd[lo:hi, kt * 128:(kt + 1) * 128],
                qtp[lo:hi, f, qc * 512:(qc + 1) * 512])
        nc.scalar.activation(
            expt[:, 2 * kt2:2 * kt2 + 2, :], ps[:], AF.Exp, scale=SCALE)

    # cn tiles buffer the normalized ctx of BOTH kv halves of an ftile
    # ([128 q, 128] = head f in cols 0:64, head f+4 in 64:128) so a single
    # XBAR DMA transpose produces the full-partition ctxT block — no PE
    # transpose, no DVE psum->sbuf copy.
    cns = {}

    def ctx_sub(qc, hd, sub, expt, cT):
        u, f = hd // 4, hd % 4

        def chain():
            pc = psum.tile([128, DH + 1], F32, tag="cx", bufs=2,
                           name=f"pc_{qc}_{hd}_{sub}")
            for kt in range(NKT):
                nc.tensor.matmul(
                    pc[:], expt[:, kt, sub * 128:(sub + 1) * 128],
                    vsb[:, kt, u * 66:u * 66 + DH + 1],
                    start=(kt == 0), stop=(kt == NKT - 1))
            rc = smal.tile([128, 1], F32, tag="rc", name=f"rc_{qc}_{hd}_{sub}")
            nc.vector.reciprocal(rc[:], pc[:, DH:DH + 1])
            if (qc, f, sub) not in cns:
                cns[(qc, f, sub)] = smal.tile(
                    [128, 128], BF16, tag="cn", bufs=20,
                    name=f"cn_{qc}_{f}_{sub}")
            cn = cns[(qc, f, sub)]
            nc.vector.tensor_scalar_mul(
                cn[:, u * DH:(u + 1) * DH], pc[:, 0:DH], rc[:])
            if u == 1:
                del cns[(qc, f, sub)]

                def transp():
                    nc.sync.dma_start_transpose(
                        cT[:, f, sub * 128:(sub + 1) * 128], cn[:])

                # near the FRONT (slightly deferred): must stay ahead of any
                # out-proj filler that reads this ctxT subtile.
                fillers.insert(min(2, len(fillers)),
                               (60.0, transp, f"ctx{qc}"))

        return chain

    def out_chunk(qc, sub, ch, cT, ost, tail=False):
        po = psum.tile([128, 512], F32, tag="mm", bufs=2,
                       name=f"po_{qc}_{sub}_{ch}")
        for i in range(4):
            nc.tensor.matmul(
                po[:], cT[:, i, sub * 128:(sub + 1) * 128],
                wo_sb[:, i, ch * 512:(ch + 1) * 512],
                start=(i == 0), stop=(i == 3))
        if tail and ch % 2 == 0:
            nc.scalar.copy(ost[:, ch * 512:(ch + 1) * 512], po[:])
        else:
            nc.vector.tensor_copy(ost[:, ch * 512:(ch + 1) * 512], po[:])
        rows = slice(qc * 512 + sub * 128, qc * 512 + (sub + 1) * 128)
        if tail:
            cols = slice(ch * 512, (ch + 1) * 512)
            nc.sync.dma_start(out[rows, cols], ost[:, cols])
        elif ch % 2 == 1:
            cols = slice((ch - 1) * 512, (ch + 1) * 512)
            nc.sync.dma_start(out[rows, cols], ost[:, cols])

    prev = None            # (qc, ctxT) pending output projection
    pend = []              # [(qc, hd, expt, ctxT)] ctx not yet enqueued
    # out-proj chunks flow through a global FIFO, drained unevenly across
    # the later qcs (qc3 has no next-qc Q-proj fillers, so it gets more).
    outq = deque()
    osts = {}

    def push_out_chunks(pqc, pcT):
        for sub in range(4):
            for ch in range(4):
                def mk(s=sub, c=ch, t=pcT, q=pqc):
                    if (q, s) not in osts:
                        osts[(q, s)] = ctxp.tile(
                            [128, DM], BF16, tag="ost", bufs=2,
                            name=f"ost_{q}_{s}")
                    out_chunk(q, s, c, t, osts[(q, s)])
                outq.append(mk)

    OUT_PER_HEAD = {
        1: [2, 1, 2, 1, 2, 1, 2, 1],
        2: [2, 1, 2, 1, 2, 1, 2, 1],
        3: [2, 2, 3, 3, 3, 3, 4, 4],
    }

    def warm_fill(n, tag):
        # idle-filler matmuls in Act-starved stretches: keep the PE clock
        # ramped so the next real chain dispatches at the fast rate
        for _ in range(n):
            def wf():
                pw = psum.tile([128, DH + 1], F32, tag="cx", bufs=2,
                               name=f"warmf_{warmup.i}")
                warmup.i += 1
                nc.tensor.matmul(pw[:], warm[:], warm[:, 0:DH + 1])
            fillers.append((110.0, wf, tag))

    for qc in range(4):
        ctxT = ctxp.tile([128, 4, 512], BF16, tag="ctxT", name=f"ctxT_{qc}")
        defer = 3 if qc < 3 else 1
        # stream next q chunk (and wo after the last input chunk)
        q_next = dma_q(qc + 1) if qc < 3 else None
        if qc == 0:
            nc.sync.dma_start(wo_sb[:], wo.rearrange("(i p) d -> p i d", p=128))
        # order barrier: qtp(qc) writes must be emitted before qc's pairs
        if qc > 0:
            drain_tag(f"qp{qc}")
        for hd in range(8):
            if qc == 0 and hd < 4:
                for t in range(4 * hd, 4 * hd + 4):
                    fillers.append((860.0, lambda t=t: v_proj(t), "vp"))
            # at qc start flush ALL pending ctx (so out fillers of the
            # previous qc are enqueued after every ctx write of that qc)
            want = 0 if hd == 0 else defer
            while len(pend) > want:
                pq2, phd, pexp, pcT2 = pend.pop(0)
                for sub in range(4):
                    fillers.append(
                        (440.0, ctx_sub(pq2, phd, sub, pexp, pcT2),
                         f"ctx{pq2}"))
            if hd == 0 and prev is not None:
                push_out_chunks(prev[0], prev[1])
            if qc > 0:
                for _ in range(OUT_PER_HEAD[qc][hd]):
                    if outq:
                        fillers.append((860.0, outq.popleft(), "out"))

            if qc < 3 and hd >= 4:
                fillers.append(
                    (3400.0, lambda f=hd - 4, qn=q_next, qc1=qc + 1: (
                        q_proj_one(qc1, qn, f)), f"qp{qc + 1}"))
            expt = expp.tile([128, NKT, 512], BF16, tag="exp",
                             name=f"exp_{qc}_{hd}")
            for kt2 in range(NKT // 2):
                pair(qc, hd, kt2, expt)
                drain(PAIR_NS)
            pend.append((qc, hd, expt, ctxT))
        prev = (qc, ctxT)

    # tail: remaining ctx heads, fillers, final output projection
    while pend:
        pq2, phd, pexp, pcT2 = pend.pop(0)
        for sub in range(4):
            ctx_sub(pq2, phd, sub, pexp, pcT2)()
            drain(PAIR_NS)
    drain(1e12)
    pqc, pcT = prev
    for sub in range(4):
        ost = ctxp.tile([128, DM], BF16, tag="ost", bufs=2,
                        name=f"ost_{pqc}_{sub}")
        for ch in range(4):
            out_chunk(pqc, sub, ch, pcT, ost, tail=True)


def _build():
    nc = bacc.Bacc("TRN2", target_bir_lowering=False, debug=False, num_devices=NC)
    qT = nc.dram_tensor("qT", [DM, S], BF16, kind="ExternalInput")
    kT = nc.dram_tensor("kT", [DM, S], BF16, kind="ExternalInput")
    vT = nc.dram_tensor("vT", [DM, S], BF16, kind="ExternalInput")
    # weights pre-arranged on host for contiguous-per-partition DMA
    wq = nc.dram_tensor("wq", [128, 4 * DT * 128], BF16, kind="ExternalInput")
    wk = nc.dram_tensor("wk", [128, DT * DKV], BF16, kind="ExternalInput")
    wv = nc.dram_tensor("wv", [128, DT * DKV], BF16, kind="ExternalInput")
    wo = nc.dram_tensor("wo", [DQ, DM], BF16, kind="ExternalInput")
    out = nc.dram_tensor("out", [S, DM], BF16, kind="ExternalOutput")
    with tile.TileContext(nc) as tc:
        with ExitStack() as ctx:
            _emit(ctx, tc, qT.ap(), kT.ap(), vT.ap(), wq.ap(), wk.ap(),
                  wv.ap(), wo.ap(), out.ap())
    nc.compile()
    return nc


def _make_runner(nc, n_cores=NC):
    """Build the sharded jit callable once; reuse across kernel() calls."""
    bass2jax.install_neuronx_cc_hook()
    partition_name = nc.partition_id_tensor.name if nc.partition_id_tensor else None
    in_names, out_names, out_avals, zero_outs = [], [], [], []
    for alloc in nc.m.functions[0].allocations:
        if not isinstance(alloc, mybir.MemoryLocationSet):
            continue
        name = alloc.memorylocations[0].name
        if alloc.kind == "ExternalInput":
            if name != partition_name:
                in_names.append(name)
        elif alloc.kind == "ExternalOutput":
            out_names.append(name)
            shape = tuple(alloc.tensor_shape)
            dtype = mybir.dt.np(alloc.dtype)
            out_avals.append(jax.core.ShapedArray(shape, dtype))
            zero_outs.append(np.zeros(shape, dtype))
    n_params = len(in_names)
    n_outs = len(out_avals)
    in_names_all = in_names + out_names
    if partition_name is not None:
        in_names_all.append(partition_name)
    donate = tuple(range(n_params, n_params + n_outs))

    def _body(*args):
        operands = list(args)
        if partition_name is not None:
            operands.append(bass2jax.partition_id_tensor())
        outs = bass2jax._bass_exec_p.bind(
            *operands,
            out_avals=tuple(out_avals),
            in_names=tuple(in_names_all),
            out_names=tuple(out_names),
            lowering_input_output_aliases=(),
            sim_require_finite=True,
            sim_require_nnan=True,
            nc=nc,
        )
        return tuple(outs)

    devices = jax.devices()[:n_cores]
    mesh = Mesh(np.asarray(devices), ("core",))
    in_specs = (PartitionSpec("core"),) * (n_params + n_outs)
    out_specs = (PartitionSpec("core"),) * len(out_names)
    sharded = jax.jit(
        shard_map(_body, mesh=mesh, in_specs=in_specs, out_specs=out_specs,
                  check_rep=False),
        donate_argnums=donate, keep_unused=True)
    sh = NamedSharding(mesh, PartitionSpec("core"))
    return sharded, in_names, out_names, zero_outs, sh


def _run(in_maps):
    if "nc" not in _cache:
        _cache["nc"] = _build()
    if "runner" not in _cache:
        _cache["runner"] = _make_runner(_cache["nc"])
    sharded, in_names, out_names, zero_outs, sh = _cache["runner"]
    n = NC
    concat_in = [
        jax.device_put(
            np.concatenate([np.asarray(in_maps[c][nm]) for c in range(n)], 0), sh)
        for nm in in_names
    ]
    zeros = [
        jax.device_put(np.zeros((n * z.shape[0], *z.shape[1:]), z.dtype), sh)
        for z in zero_outs
    ]
    outs = sharded(*concat_in, *zeros)
    i = out_names.index("out")
    arr = np.asarray(outs[i])           # [NC*S, DM]
    return arr.reshape(n, S, DM)


def kernel(q, k, v, Wq, Wk, Wv, Wo):
    q = np.asarray(q, dtype=np.float32)
    k = np.asarray(k, dtype=np.float32)
    v = np.asarray(v, dtype=np.float32)
    bf = ml_dtypes.bfloat16
    qTh = np.ascontiguousarray(q.astype(bf).transpose(0, 2, 1))
    kTh = np.ascontiguousarray(k.astype(bf).transpose(0, 2, 1))
    vTh = np.ascontiguousarray(v.astype(bf).transpose(0, 2, 1))
    Wq32 = np.asarray(Wq, dtype=np.float32)
    Wk32 = np.asarray(Wk, dtype=np.float32)
    Wv32 = np.asarray(Wv, dtype=np.float32)
    Wo32 = np.asarray(Wo, dtype=np.float32)

    in_maps = []
    for c in range(NC):
        b, g = c // 4, c % 4
        # head (u, f) -> qtp ftile f, rows u*64:(u+1)*64
        wq_s = Wq32[:, g * DQ:(g + 1) * DQ].reshape(DM, 2, 4, DH)
        wq_p = wq_s.transpose(0, 2, 1, 3).reshape(DM, DQ)
        # device layout [p, f, dt, d]: wq_pre[p, f*DT*128 + dt*128 + x]
        # = wq_p[dt*128 + p, f*128 + x]
        wq_pre = (wq_p.reshape(DT, 128, 4, 128)
                  .transpose(1, 2, 0, 3).reshape(128, 4 * DT * 128))
        # device layout [p, dt, m]: w_pre[p, dt*DKV + m] = w[dt*128 + p, m]
        wk_s = Wk32[:, g * DKV:(g + 1) * DKV]
        wk_pre = wk_s.reshape(DT, 128, DKV).transpose(1, 0, 2).reshape(128, -1)
        wv_s = Wv32[:, g * DKV:(g + 1) * DKV]
        wv_pre = wv_s.reshape(DT, 128, DKV).transpose(1, 0, 2).reshape(128, -1)
        wo_s = Wo32[g * DQ:(g + 1) * DQ, :].reshape(2, 4, DH, DM)
        wo_p = wo_s.transpose(1, 0, 2, 3).reshape(DQ, DM)
        in_maps.append({
            "qT": qTh[b], "kT": kTh[b], "vT": vTh[b],
            "wq": np.ascontiguousarray(wq_pre.astype(bf)),
            "wk": np.ascontiguousarray(wk_pre.astype(bf)),
            "wv": np.ascontiguousarray(wv_pre.astype(bf)),
            "wo": np.ascontiguousarray(wo_p.astype(bf)),
        })
    partials = _run(in_maps)            # [8, S, DM] bf16
    out = partials.astype(np.float32, copy=False).reshape(2, 4, S, DM).sum(axis=1)
    return out


# revision 7
# speedup vs baseline: 1.0147x; 1.0047x over previous
"""GQA multi-head attention (B=2, S=2048, D=2048, 32 q-heads / 8 kv-heads)
on 8 Trainium2 NeuronCores.

Sharding: batch (2-way) x kv-head-pair (4-way). Core c owns batch c//4 and
kv heads {2g, 2g+1} with g = c%4, i.e. q heads 8g..8g+7 (512 q-proj feats).
Wq/Wk/Wv column-sharded, Wo row-sharded; host sums the 4 partials per batch.

Per-core dataflow (bf16 operands, fp32 PSUM):
  K^T [128f, S]  = Wk_c^T @ k^T          (2 kv heads fill all 128 partitions)
  V   [S, 128f]  = v-tiles^T... via lhsT=vT chunk, rhs=Wv_c (tokens on
                   partitions; +ones cols for softmax denominators)
  Q^T [128, 4, S]: ftile f rows u*64:(u+1)*64 = head (u,f) (u = kv half)
  S^T [kv, q] = ktd[u-half].T-slice @ qtp[u-half, f]   (64-contraction)
  expS = exp(S^T * 1/8) on ACT -> bf16 SBUF
  ctx [q, 65] = expS^T-chunk.T @ V_aug   (ones col -> rowsum at col 64)
  cnorm = ctx[:,0:64] * recip(ctx[:,64]) (DVE per-partition scalar mul)
  ctxT via PE transpose; out [q, dm] = ctxT.T @ Wo_c -> DMA fp32 psum->HBM
"""

from contextlib import ExitStack

import numpy as np
import ml_dtypes

import jax

try:
    jax.config.update("jax_compilation_cache_dir", "/tmp/jax_bass_cache")
    jax.config.update("jax_persistent_cache_min_compile_time_secs", 1.0)
except Exception:
    pass

from jax.sharding import Mesh, PartitionSpec, NamedSharding
from jax.experimental.shard_map import shard_map

import concourse.bass as bass
import concourse.mybir as mybir
import concourse.tile as tile
from concourse import bacc, bass2jax

BF16 = mybir.dt.bfloat16
F32 = mybir.dt.float32
AF = mybir.ActivationFunctionType

B, S, DM = 2, 2048, 2048
NC = 8
DT = DM // 128         # 16 contraction tiles
DH = 64
DQ = 512               # per-core q-proj width (8 heads)
DKV = 128              # per-core kv width (2 heads)
NKT = S // 128         # 16 kv tiles
SCALE = 1.0 / 8.0      # 1/sqrt(64)

_cache = {}


def _emit(ctx, tc, qT, kT, vT, wq, wk, wv, wo, out):
    nc = tc.nc

    pp = ctx.enter_context(tc.tile_pool(name="persist", bufs=1))
    wq_sb = pp.tile([128, 4, DT, 128], BF16, tag="wq")   # [p, f, dt, d]
    wk_sb = pp.tile([128, DT, DKV], BF16, tag="wk")
    wv_sb = pp.tile([128, DT, DKV], BF16, tag="wv")
    wo_sb = pp.tile([128, 4, DM], BF16, tag="wo")
    qtp = pp.tile([128, 4, S], BF16, tag="qtp")      # [u*64+d, f, q]
    ktd = pp.tile([128, S], BF16, tag="ktd")         # [u*64+d, kv]
    vsb = pp.tile([128, NKT, 132], BF16, tag="vsb")  # [kv%128, kvtile, 2x66]
    ident = pp.tile([128, 128], BF16, tag="ident")

    from concourse.masks import make_identity
    make_identity(nc, ident[:])
    nc.gpsimd.memset(vsb[:, :, DH], 1.0)        # ones col, kv head 0
    nc.gpsimd.memset(vsb[:, :, 66 + DH], 1.0)   # ones col, kv head 1
    warm = pp.tile([128, 128], BF16, tag="warm")  # also feeds warm_fill

    stage = ctx.enter_context(tc.tile_pool(name="stage", bufs=3))
    expp = ctx.enter_context(tc.tile_pool(name="expp", bufs=4))
    smal = ctx.enter_context(tc.tile_pool(name="small", bufs=3))
    ctxp = ctx.enter_context(tc.tile_pool(name="ctxp", bufs=2))
    psum = ctx.enter_context(tc.tile_pool(name="psum", bufs=1, space="PSUM"))

    # ---------------- staged input DMA helpers ----------------
    def dma_k(c):
        k_ch = stage.tile([128, DT, 512], BF16, tag="instage", name=f"k_ch_{c}")
        nc.sync.dma_start(
            k_ch[:],
            kT.rearrange("(dt p) s -> p dt s", p=128)[:, :, c * 512:(c + 1) * 512])
        return k_ch

    def dma_q(qc):
        q_ch = stage.tile([128, DT, 512], BF16, tag="instage", name=f"q_ch_{qc}")
        qv = qT.rearrange("(dt p) s -> p dt s", p=128)[:, :, qc * 512:(qc + 1) * 512]
        for qq in range(4):
            nc.sync.dma_start(q_ch[:, qq * 4:qq * 4 + 4], qv[:, qq * 4:qq * 4 + 4])
        return q_ch

    def dma_v(c):
        v_ch = stage.tile([128, DT, 512], BF16, tag="instage", name=f"v_ch_{c}")
        vv = vT.rearrange("(dt p) s -> p dt s", p=128)[:, :, c * 512:(c + 1) * 512]
        # quarters by dt: V-proj chains begin accumulating on the first
        # quarter while the rest streams (qc0 is DMA-starved here)
        for qq in range(4):
            nc.sync.dma_start(v_ch[:, qq * 4:qq * 4 + 4], vv[:, qq * 4:qq * 4 + 4])
        return v_ch

    # Weights arrive pre-arranged on the host so every DMA has >=512B
    # contiguous runs (full DMA rate). K streams in 5 chunks, narrow first,
    # so K-proj and the first score pairs start as early as possible.
    K_CH = [(0, 256), (256, 256), (512, 512), (1024, 512), (1536, 512)]

    def dma_kc(i):
        off, w = K_CH[i]
        k_ch = stage.tile([128, DT, w], BF16, tag="instage", name=f"k_ch_{i}")
        kv = kT.rearrange("(dt p) s -> p dt s", p=128)[:, :, off:off + w]
        if w >= 512:
            for qq in range(4):
                nc.sync.dma_start(k_ch[:, qq * 4:qq * 4 + 4], kv[:, qq * 4:qq * 4 + 4])
        else:
            nc.sync.dma_start(k_ch[:], kv)
        return k_ch

    nc.sync.dma_start(wk_sb[:], wk.rearrange("p (dt m) -> p dt m", m=DKV))
    k_chs = [dma_kc(0)]
    wqr = wq.rearrange("p (f dt m) -> p f dt m", f=4, m=128)
    nc.sync.dma_start(wq_sb[:, 0], wqr[:, 0])
    q_ch0 = dma_q(0)
    k_chs.append(dma_kc(1))
    nc.sync.dma_start(wq_sb[:, 1:4], wqr[:, 1:4])
    nc.sync.dma_start(wv_sb[:], wv.rearrange("p (dt m) -> p dt m", m=DKV))
    k_chs += [dma_kc(2), dma_kc(3), dma_kc(4)]
    v_chs = [dma_v(c) for c in range(4)]

    def k_proj(c):
        off, w = K_CH[c]
        pk = psum.tile([128, w], F32, tag="mm", bufs=2, name=f"pk_{c}")
        for dt in range(DT):
            nc.tensor.matmul(pk[:], wk_sb[:, dt, :], k_chs[c][:, dt, :],
                             start=(dt == 0), stop=(dt == DT - 1))
        nc.vector.tensor_copy(ktd[:, off:off + w], pk[:])

    def v_proj(t):
        # demoted priority: these chains unblock when their vT chunk lands
        # mid-attention; without the demotion they preempt the score pairs
        # that feed ACT (priority inversion -> ACT starvation)
        with tc.high_priority(-400):
            pv = psum.tile([128, DKV], F32, tag="mm", bufs=2, name=f"pv_{t}")
            for dt in range(DT):
                nc.tensor.matmul(
                    pv[:], v_chs[t // 4][:, dt, (t % 4) * 128:(t % 4 + 1) * 128],
                    wv_sb[:, dt, :], start=(dt == 0), stop=(dt == DT - 1))
            # [:, 0:64] -> cols 0:64 (head 0), [:, 64:128] -> cols 66:130
            dst = vsb[:, t, :].rearrange("p (a b) -> p a b", b=66)[:, :, 0:DH]
            nc.vector.tensor_copy(dst, pv[:].rearrange("p (a b) -> p a b", b=DH))

    def q_chain_block(pq, q_ch, f, quarter):
        """4 of the 16 accumulation steps of Q-proj chain f."""
        for dt in range(quarter * 4, quarter * 4 + 4):
            nc.tensor.matmul(
                pq[:], wq_sb[:, f, dt, :], q_ch[:, dt, :],
                start=(dt == 0), stop=(dt == DT - 1))

    from contextlib import contextmanager

    @contextmanager
    def _null():
        yield

    def q_proj_one(qc, q_ch, f, demote=0):
        with tc.high_priority(-demote) if demote else _null():
            pq = psum.tile([128, 512], F32, tag="mm", bufs=2,
                           name=f"pq_{qc}_{f}")
            for quarter in range(4):
                q_chain_block(pq, q_ch, f, quarter)
            nc.vector.tensor_copy(qtp[:, f, qc * 512:(qc + 1) * 512], pq[:])

    # Warm-up matmuls: keep the PE busy (and its p-state ramping toward full
    # clock) while the first input DMAs stream in. Results are never read.
    nc.vector.memset(warm[:], 0.0)

    def warmup(n):
        # Keep the PE continuously busy (matmul p-state stays ramped) while
        # input DMAs stream in; results are never read. Priority order lets
        # real work preempt the remaining warmups the moment it is ready.
        for w in range(n):
            pw = psum.tile([128, 2, 128], F32, tag="sc", bufs=2,
                           name=f"warm_{warmup.i}")
            warmup.i += 1
            nc.tensor.matmul(pw[:, 0], warm[:], warm[:])
            nc.tensor.matmul(pw[:, 1], warm[:], warm[:])
    warmup.i = 0

    # PE preamble: narrow K chunks and Q proj(0) interleaved with arrival
    # order; V proj chains are deferred into the qc0 filler stream. Warmups
    # interleave so the PE never idles (a PE idle gap resets the clock ramp
    # and the next chain would dispatch at the cold rate).
    warmup(25)
    k_proj(0)
    warmup(34)
    q_proj_one(0, q_ch0, 0)
    k_proj(1)
    for f in range(1, 4):
        q_proj_one(0, q_ch0, f)
    for c in range(2, 5):
        k_proj(c)

    # ---------------- pipelined attention + output projection ----------------
    # Score-pair emissions are the clock; between pairs we drain ~PAIR_NS of
    # deferred PE micro-tasks (ctx chains, transposes, out-proj chunks,
    # quarter Q-proj chains, V-proj chains) from a FIFO.
    from collections import deque
    fillers = deque()   # (cost_ns, thunk, tag)
    PAIR_NS = 700.0

    def drain(budget):
        got = 0.0
        while fillers and got < budget:
            cost, thunk, _ = fillers.popleft()
            thunk()
            got += cost

    def drain_tag(tag):
        """Emit (FIFO) until no filler with `tag` remains — order barrier."""
        while any(t == tag for _, _, t in fillers):
            cost, thunk, _ = fillers.popleft()
            thunk()

    def pair(qc, hd, kt2, expt):
        u, f = hd // 4, hd % 4
        lo, hi = u * DH, (u + 1) * DH
        ps = psum.tile([128, 2, 512], F32, tag="sc", bufs=2,
                       name=f"ps_{qc}_{hd}_{kt2}")
        for j in range(2):
            kt = 2 * kt2 + j
            nc.tensor.matmul(
                ps[:, j, :],
                ktd[lo:hi, kt * 128:(kt + 1) * 128],
                qtp[lo:hi, f, qc * 512:(qc + 1) * 512])
        nc.scalar.activation(
            expt[:, 2 * kt2:2 * kt2 + 2, :], ps[:], AF.Exp, scale=SCALE)

    # cn tiles buffer the normalized ctx of BOTH kv halves of an ftile
    # ([128 q, 128] = head f in cols 0:64, head f+4 in 64:128) so a single
    # XBAR DMA transpose produces the full-partition ctxT block — no PE
    # transpose, no DVE psum->sbuf copy.
    cns = {}

    def ctx_sub(qc, hd, sub, expt, cT):
        u, f = hd // 4, hd % 4

        def chain():
            pc = psum.tile([128, DH + 1], F32, tag="cx", bufs=2,
                           name=f"pc_{qc}_{hd}_{sub}")
            for kt in range(NKT):
                nc.tensor.matmul(
                    pc[:], expt[:, kt, sub * 128:(sub + 1) * 128],
                    vsb[:, kt, u * 66:u * 66 + DH + 1],
                    start=(kt == 0), stop=(kt == NKT - 1))
            rc = smal.tile([128, 1], F32, tag="rc", name=f"rc_{qc}_{hd}_{sub}")
            nc.vector.reciprocal(rc[:], pc[:, DH:DH + 1])
            if (qc, f, sub) not in cns:
                cns[(qc, f, sub)] = smal.tile(
                    [128, 128], BF16, tag="cn", bufs=20,
                    name=f"cn_{qc}_{f}_{sub}")
            cn = cns[(qc, f, sub)]
            nc.vector.tensor_scalar_mul(
                cn[:, u * DH:(u + 1) * DH], pc[:, 0:DH], rc[:])
            if u == 1:
                del cns[(qc, f, sub)]

                def transp():
                    nc.sync.dma_start_transpose(
                        cT[:, f, sub * 128:(sub + 1) * 128], cn[:])

                # near the FRONT (slightly deferred): must stay ahead of any
                # out-proj filler that reads this ctxT subtile.
                fillers.insert(min(2, len(fillers)),
                               (60.0, transp, f"ctx{qc}"))

        return chain

    def out_chunk(qc, sub, ch, cT, ost, tail=False):
        # tail chunks also rotate through the cx tag (idle after the last
        # ctx chain) for a 4-deep psum rotation at the end
        po = psum.tile([128, 512], F32,
                       tag=("cx" if tail and ch % 2 else "mm"), bufs=2,
                       name=f"po_{qc}_{sub}_{ch}")
        for i in range(4):
            nc.tensor.matmul(
                po[:], cT[:, i, sub * 128:(sub + 1) * 128],
                wo_sb[:, i, ch * 512:(ch + 1) * 512],
                start=(i == 0), stop=(i == 3))
        if tail and ch % 2 == 0:
            nc.scalar.copy(ost[:, ch * 512:(ch + 1) * 512], po[:])
        else:
            nc.vector.tensor_copy(ost[:, ch * 512:(ch + 1) * 512], po[:])
        rows = slice(qc * 512 + sub * 128, qc * 512 + (sub + 1) * 128)
        if tail:
            cols = slice(ch * 512, (ch + 1) * 512)
            nc.sync.dma_start(out[rows, cols], ost[:, cols])
        elif ch % 2 == 1:
            cols = slice((ch - 1) * 512, (ch + 1) * 512)
            nc.sync.dma_start(out[rows, cols], ost[:, cols])

    prev = None            # (qc, ctxT) pending output projection
    pend = []              # [(qc, hd, expt, ctxT)] ctx not yet enqueued
    # out-proj chunks flow through a global FIFO, drained unevenly across
    # the later qcs (qc3 has no next-qc Q-proj fillers, so it gets more).
    outq = deque()
    osts = {}

    def push_out_chunks(pqc, pcT):
        for sub in range(4):
            for ch in range(4):
                def mk(s=sub, c=ch, t=pcT, q=pqc):
                    if (q, s) not in osts:
                        osts[(q, s)] = ctxp.tile(
                            [128, DM], BF16, tag="ost", bufs=2,
                            name=f"ost_{q}_{s}")
                    out_chunk(q, s, c, t, osts[(q, s)])
                outq.append(mk)

    OUT_PER_HEAD = {
        1: [2, 1, 2, 1, 2, 1, 2, 1],
        2: [2, 1, 2, 1, 2, 1, 2, 1],
        3: [2, 2, 3, 3, 3, 3, 4, 4],
    }

    def warm_fill(n, tag):
        # idle-filler matmuls in Act-starved stretches: keep the PE clock
        # ramped so the next real chain dispatches at the fast rate
        for _ in range(n):
            def wf():
                pw = psum.tile([128, DH + 1], F32, tag="cx", bufs=2,
                               name=f"warmf_{warmup.i}")
                warmup.i += 1
                nc.tensor.matmul(pw[:], warm[:], warm[:, 0:DH + 1])
            fillers.append((110.0, wf, tag))

    for qc in range(4):
        ctxT = ctxp.tile([128, 4, 512], BF16, tag="ctxT", name=f"ctxT_{qc}")
        defer = 3 if qc < 3 else 1
        # stream next q chunk (and wo after the last input chunk)
        q_next = dma_q(qc + 1) if qc < 3 else None
        if qc == 0:
            nc.sync.dma_start(wo_sb[:], wo.rearrange("(i p) d -> p i d", p=128))
        # order barrier: qtp(qc) writes must be emitted before qc's pairs
        if qc > 0:
            drain_tag(f"qp{qc}")
        for hd in range(8):
            if qc == 0 and hd < 4:
                for t in range(4 * hd, 4 * hd + 4):
                    fillers.append((860.0, lambda t=t: v_proj(t), "vp"))
            # at qc start flush ALL pending ctx (so out fillers of the
            # previous qc are enqueued after every ctx write of that qc)
            want = 0 if hd == 0 else defer
            while len(pend) > want:
                pq2, phd, pexp, pcT2 = pend.pop(0)
                for sub in range(4):
                    fillers.append(
                        (440.0, ctx_sub(pq2, phd, sub, pexp, pcT2),
                         f"ctx{pq2}"))
            if hd == 0 and prev is not None:
                push_out_chunks(prev[0], prev[1])
            if qc > 0:
                for _ in range(OUT_PER_HEAD[qc][hd]):
                    if outq:
                        fillers.append((860.0, outq.popleft(), "out"))

            if qc < 3 and hd >= 4:
                fillers.append(
                    (3400.0, lambda f=hd - 4, qn=q_next, qc1=qc + 1: (
                        q_proj_one(qc1, qn, f)), f"qp{qc + 1}"))
            expt = expp.tile([128, NKT, 512], BF16, tag="exp",
                             name=f"exp_{qc}_{hd}")
            for kt2 in range(NKT // 2):
                pair(qc, hd, kt2, expt)
                drain(PAIR_NS)
            pend.append((qc, hd, expt, ctxT))
        prev = (qc, ctxT)

    # tail: remaining ctx heads, fillers, final output projection
    while pend:
        pq2, phd, pexp, pcT2 = pend.pop(0)
        for sub in range(4):
            ctx_sub(pq2, phd, sub, pexp, pcT2)()
            drain(PAIR_NS)
    drain(1e12)
    pqc, pcT = prev
    for sub in range(4):
        ost = ctxp.tile([128, DM], BF16, tag="ost", bufs=2,
                        name=f"ost_{pqc}_{sub}")
        for ch in range(4):
            out_chunk(pqc, sub, ch, pcT, ost, tail=True)


def _build():
    nc = bacc.Bacc("TRN2", target_bir_lowering=False, debug=False, num_devices=NC)
    qT = nc.dram_tensor("qT", [DM, S], BF16, kind="ExternalInput")
    kT = nc.dram_tensor("kT", [DM, S], BF16, kind="ExternalInput")
    vT = nc.dram_tensor("vT", [DM, S], BF16, kind="ExternalInput")
    # weights pre-arranged on host for contiguous-per-partition DMA
    wq = nc.dram_tensor("wq", [128, 4 * DT * 128], BF16, kind="ExternalInput")
    wk = nc.dram_tensor("wk", [128, DT * DKV], BF16, kind="ExternalInput")
    wv = nc.dram_tensor("wv", [128, DT * DKV], BF16, kind="ExternalInput")
    wo = nc.dram_tensor("wo", [DQ, DM], BF16, kind="ExternalInput")
    out = nc.dram_tensor("out", [S, DM], BF16, kind="ExternalOutput")
    with tile.TileContext(nc) as tc:
        with ExitStack() as ctx:
            _emit(ctx, tc, qT.ap(), kT.ap(), vT.ap(), wq.ap(), wk.ap(),
                  wv.ap(), wo.ap(), out.ap())
    nc.compile()
    return nc


def _make_runner(nc, n_cores=NC):
    """Build the sharded jit callable once; reuse across kernel() calls."""
    bass2jax.install_neuronx_cc_hook()
    partition_name = nc.partition_id_tensor.name if nc.partition_id_tensor else None
    in_names, out_names, out_avals, zero_outs = [], [], [], []
    for alloc in nc.m.functions[0].allocations:
        if not isinstance(alloc, mybir.MemoryLocationSet):
            continue
        name = alloc.memorylocations[0].name
        if alloc.kind == "ExternalInput":
            if name != partition_name:
                in_names.append(name)
        elif alloc.kind == "ExternalOutput":
            out_names.append(name)
            shape = tuple(alloc.tensor_shape)
            dtype = mybir.dt.np(alloc.dtype)
            out_avals.append(jax.core.ShapedArray(shape, dtype))
            zero_outs.append(np.zeros(shape, dtype))
    n_params = len(in_names)
    n_outs = len(out_avals)
    in_names_all = in_names + out_names
    if partition_name is not None:
        in_names_all.append(partition_name)
    donate = tuple(range(n_params, n_params + n_outs))

    def _body(*args):
        operands = list(args)
        if partition_name is not None:
            operands.append(bass2jax.partition_id_tensor())
        outs = bass2jax._bass_exec_p.bind(
            *operands,
            out_avals=tuple(out_avals),
            in_names=tuple(in_names_all),
            out_names=tuple(out_names),
            lowering_input_output_aliases=(),
            sim_require_finite=True,
            sim_require_nnan=True,
            nc=nc,
        )
        return tuple(outs)

    devices = jax.devices()[:n_cores]
    mesh = Mesh(np.asarray(devices), ("core",))
    in_specs = (PartitionSpec("core"),) * (n_params + n_outs)
    out_specs = (PartitionSpec("core"),) * len(out_names)
    sharded = jax.jit(
        shard_map(_body, mesh=mesh, in_specs=in_specs, out_specs=out_specs,
                  check_rep=False),
        donate_argnums=donate, keep_unused=True)
    sh = NamedSharding(mesh, PartitionSpec("core"))
    return sharded, in_names, out_names, zero_outs, sh


def _run(in_maps):
    if "nc" not in _cache:
        _cache["nc"] = _build()
    if "runner" not in _cache:
        _cache["runner"] = _make_runner(_cache["nc"])
    sharded, in_names, out_names, zero_outs, sh = _cache["runner"]
    n = NC
    concat_in = [
        jax.device_put(
            np.concatenate([np.asarray(in_maps[c][nm]) for c in range(n)], 0), sh)
        for nm in in_names
    ]
    zeros = [
        jax.device_put(np.zeros((n * z.shape[0], *z.shape[1:]), z.dtype), sh)
        for z in zero_outs
    ]
    outs = sharded(*concat_in, *zeros)
    i = out_names.index("out")
    arr = np.asarray(outs[i])           # [NC*S, DM]
    return arr.reshape(n, S, DM)


def kernel(q, k, v, Wq, Wk, Wv, Wo):
    q = np.asarray(q, dtype=np.float32)
    k = np.asarray(k, dtype=np.float32)
    v = np.asarray(v, dtype=np.float32)
    bf = ml_dtypes.bfloat16
    qTh = np.ascontiguousarray(q.astype(bf).transpose(0, 2, 1))
    kTh = np.ascontiguousarray(k.astype(bf).transpose(0, 2, 1))
    vTh = np.ascontiguousarray(v.astype(bf).transpose(0, 2, 1))
    Wq32 = np.asarray(Wq, dtype=np.float32)
    Wk32 = np.asarray(Wk, dtype=np.float32)
    Wv32 = np.asarray(Wv, dtype=np.float32)
    Wo32 = np.asarray(Wo, dtype=np.float32)

    in_maps = []
    for c in range(NC):
        b, g = c // 4, c % 4
        # head (u, f) -> qtp ftile f, rows u*64:(u+1)*64
        wq_s = Wq32[:, g * DQ:(g + 1) * DQ].reshape(DM, 2, 4, DH)
        wq_p = wq_s.transpose(0, 2, 1, 3).reshape(DM, DQ)
        # device layout [p, f, dt, d]: wq_pre[p, f*DT*128 + dt*128 + x]
        # = wq_p[dt*128 + p, f*128 + x]
        wq_pre = (wq_p.reshape(DT, 128, 4, 128)
                  .transpose(1, 2, 0, 3).reshape(128, 4 * DT * 128))
        # device layout [p, dt, m]: w_pre[p, dt*DKV + m] = w[dt*128 + p, m]
        wk_s = Wk32[:, g * DKV:(g + 1) * DKV]
        wk_pre = wk_s.reshape(DT, 128, DKV).transpose(1, 0, 2).reshape(128, -1)
        wv_s = Wv32[:, g * DKV:(g + 1) * DKV]
        wv_pre = wv_s.reshape(DT, 128, DKV).transpose(1, 0, 2).reshape(128, -1)
        wo_s = Wo32[g * DQ:(g + 1) * DQ, :].reshape(2, 4, DH, DM)
        wo_p = wo_s.transpose(1, 0, 2, 3).reshape(DQ, DM)
        in_maps.append({
            "qT": qTh[b], "kT": kTh[b], "vT": vTh[b],
            "wq": np.ascontiguousarray(wq_pre.astype(bf)),
            "wk": np.ascontiguousarray(wk_pre.astype(bf)),
            "wv": np.ascontiguousarray(wv_pre.astype(bf)),
            "wo": np.ascontiguousarray(wo_p.astype(bf)),
        })
    partials = _run(in_maps)            # [8, S, DM] bf16
    out = partials.astype(np.float32, copy=False).reshape(2, 4, S, DM).sum(axis=1)
    return out


# revision 8
# speedup vs baseline: 1.0151x; 1.0003x over previous
"""GQA multi-head attention (B=2, S=2048, D=2048, 32 q-heads / 8 kv-heads)
on 8 Trainium2 NeuronCores.

Sharding: batch (2-way) x kv-head-pair (4-way). Core c owns batch c//4 and
kv heads {2g, 2g+1} with g = c%4, i.e. q heads 8g..8g+7 (512 q-proj feats).
Wq/Wk/Wv column-sharded, Wo row-sharded; host sums the 4 partials per batch.

Per-core dataflow (bf16 operands, fp32 PSUM):
  K^T [128f, S]  = Wk_c^T @ k^T          (2 kv heads fill all 128 partitions)
  V   [S, 128f]  = v-tiles^T... via lhsT=vT chunk, rhs=Wv_c (tokens on
                   partitions; +ones cols for softmax denominators)
  Q^T [128, 4, S]: ftile f rows u*64:(u+1)*64 = head (u,f) (u = kv half)
  S^T [kv, q] = ktd[u-half].T-slice @ qtp[u-half, f]   (64-contraction)
  expS = exp(S^T * 1/8) on ACT -> bf16 SBUF
  ctx [q, 65] = expS^T-chunk.T @ V_aug   (ones col -> rowsum at col 64)
  cnorm = ctx[:,0:64] * recip(ctx[:,64]) (DVE per-partition scalar mul)
  ctxT via PE transpose; out [q, dm] = ctxT.T @ Wo_c -> DMA fp32 psum->HBM
"""

from contextlib import ExitStack

import numpy as np
import ml_dtypes

import jax

try:
    jax.config.update("jax_compilation_cache_dir", "/tmp/jax_bass_cache")
    jax.config.update("jax_persistent_cache_min_compile_time_secs", 1.0)
except Exception:
    pass

from jax.sharding import Mesh, PartitionSpec, NamedSharding
from jax.experimental.shard_map import shard_map

import concourse.bass as bass
import concourse.mybir as mybir
import concourse.tile as tile
from concourse import bacc, bass2jax

BF16 = mybir.dt.bfloat16
F32 = mybir.dt.float32
AF = mybir.ActivationFunctionType

B, S, DM = 2, 2048, 2048
NC = 8
DT = DM // 128         # 16 contraction tiles
DH = 64
DQ = 512               # per-core q-proj width (8 heads)
DKV = 128              # per-core kv width (2 heads)
NKT = S // 128         # 16 kv tiles
SCALE = 1.0 / 8.0      # 1/sqrt(64)

_cache = {}


def _emit(ctx, tc, qT, kT, vT, wq, wk, wv, wo, out):
    nc = tc.nc

    pp = ctx.enter_context(tc.tile_pool(name="persist", bufs=1))
    wq_sb = pp.tile([128, 4, DT, 128], BF16, tag="wq")   # [p, f, dt, d]
    wk_sb = pp.tile([128, DT, DKV], BF16, tag="wk")
    wv_sb = pp.tile([128, DT, DKV], BF16, tag="wv")
    wo_sb = pp.tile([128, 4, DM], BF16, tag="wo")
    qtp = pp.tile([128, 4, S], BF16, tag="qtp")      # [u*64+d, f, q]
    ktd = pp.tile([128, S], BF16, tag="ktd")         # [u*64+d, kv]
    vsb = pp.tile([128, NKT, 132], BF16, tag="vsb")  # [kv%128, kvtile, 2x66]
    nc.gpsimd.memset(vsb[:, :, DH], 1.0)        # ones col, kv head 0
    nc.gpsimd.memset(vsb[:, :, 66 + DH], 1.0)   # ones col, kv head 1
    warm = pp.tile([128, 128], BF16, tag="warm")  # also feeds warm_fill

    stage = ctx.enter_context(tc.tile_pool(name="stage", bufs=3))
    expp = ctx.enter_context(tc.tile_pool(name="expp", bufs=4))
    smal = ctx.enter_context(tc.tile_pool(name="small", bufs=3))
    ctxp = ctx.enter_context(tc.tile_pool(name="ctxp", bufs=2))
    psum = ctx.enter_context(tc.tile_pool(name="psum", bufs=1, space="PSUM"))

    # ---------------- staged input DMA helpers ----------------
    def dma_k(c):
        k_ch = stage.tile([128, DT, 512], BF16, tag="instage", name=f"k_ch_{c}")
        nc.sync.dma_start(
            k_ch[:],
            kT.rearrange("(dt p) s -> p dt s", p=128)[:, :, c * 512:(c + 1) * 512])
        return k_ch

    def dma_q(qc):
        q_ch = stage.tile([128, DT, 512], BF16, tag="instage", name=f"q_ch_{qc}")
        qv = qT.rearrange("(dt p) s -> p dt s", p=128)[:, :, qc * 512:(qc + 1) * 512]
        for qq in range(4):
            nc.sync.dma_start(q_ch[:, qq * 4:qq * 4 + 4], qv[:, qq * 4:qq * 4 + 4])
        return q_ch

    def dma_v(c):
        v_ch = stage.tile([128, DT, 512], BF16, tag="instage", name=f"v_ch_{c}")
        vv = vT.rearrange("(dt p) s -> p dt s", p=128)[:, :, c * 512:(c + 1) * 512]
        # quarters by dt: V-proj chains begin accumulating on the first
        # quarter while the rest streams (qc0 is DMA-starved here)
        for qq in range(4):
            nc.sync.dma_start(v_ch[:, qq * 4:qq * 4 + 4], vv[:, qq * 4:qq * 4 + 4])
        return v_ch

    # Weights arrive pre-arranged on the host so every DMA has >=512B
    # contiguous runs (full DMA rate). K streams in 5 chunks, narrow first,
    # so K-proj and the first score pairs start as early as possible.
    K_CH = [(0, 256), (256, 256), (512, 512), (1024, 512), (1536, 512)]

    def dma_kc(i):
        off, w = K_CH[i]
        k_ch = stage.tile([128, DT, w], BF16, tag="instage", name=f"k_ch_{i}")
        kv = kT.rearrange("(dt p) s -> p dt s", p=128)[:, :, off:off + w]
        if w >= 512:
            for qq in range(4):
                nc.sync.dma_start(k_ch[:, qq * 4:qq * 4 + 4], kv[:, qq * 4:qq * 4 + 4])
        else:
            nc.sync.dma_start(k_ch[:], kv)
        return k_ch

    nc.sync.dma_start(wk_sb[:], wk.rearrange("p (dt m) -> p dt m", m=DKV))
    k_chs = [dma_kc(0)]
    wqr = wq.rearrange("p (f dt m) -> p f dt m", f=4, m=128)
    nc.sync.dma_start(wq_sb[:, 0], wqr[:, 0])
    q_ch0 = dma_q(0)
    k_chs.append(dma_kc(1))
    for f in range(1, 4):
        nc.sync.dma_start(wq_sb[:, f], wqr[:, f])
    nc.sync.dma_start(wv_sb[:], wv.rearrange("p (dt m) -> p dt m", m=DKV))
    k_chs += [dma_kc(2), dma_kc(3), dma_kc(4)]
    v_chs = [dma_v(c) for c in range(4)]

    def k_proj(c):
        off, w = K_CH[c]
        pk = psum.tile([128, w], F32, tag="mm", bufs=2, name=f"pk_{c}")
        for dt in range(DT):
            nc.tensor.matmul(pk[:], wk_sb[:, dt, :], k_chs[c][:, dt, :],
                             start=(dt == 0), stop=(dt == DT - 1))
        nc.vector.tensor_copy(ktd[:, off:off + w], pk[:])

    def v_proj(t):
        # demoted priority: these chains unblock when their vT chunk lands
        # mid-attention; without the demotion they preempt the score pairs
        # that feed ACT (priority inversion -> ACT starvation)
        with tc.high_priority(-400):
            pv = psum.tile([128, DKV], F32, tag="mm", bufs=2, name=f"pv_{t}")
            for dt in range(DT):
                nc.tensor.matmul(
                    pv[:], v_chs[t // 4][:, dt, (t % 4) * 128:(t % 4 + 1) * 128],
                    wv_sb[:, dt, :], start=(dt == 0), stop=(dt == DT - 1))
            # [:, 0:64] -> cols 0:64 (head 0), [:, 64:128] -> cols 66:130
            dst = vsb[:, t, :].rearrange("p (a b) -> p a b", b=66)[:, :, 0:DH]
            nc.vector.tensor_copy(dst, pv[:].rearrange("p (a b) -> p a b", b=DH))

    def q_chain_block(pq, q_ch, f, quarter):
        """4 of the 16 accumulation steps of Q-proj chain f."""
        for dt in range(quarter * 4, quarter * 4 + 4):
            nc.tensor.matmul(
                pq[:], wq_sb[:, f, dt, :], q_ch[:, dt, :],
                start=(dt == 0), stop=(dt == DT - 1))

    from contextlib import contextmanager

    @contextmanager
    def _null():
        yield

    def q_proj_one(qc, q_ch, f, demote=0):
        with tc.high_priority(-demote) if demote else _null():
            pq = psum.tile([128, 512], F32, tag="mm", bufs=2,
                           name=f"pq_{qc}_{f}")
            for quarter in range(4):
                q_chain_block(pq, q_ch, f, quarter)
            nc.vector.tensor_copy(qtp[:, f, qc * 512:(qc + 1) * 512], pq[:])

    # Warm-up matmuls: keep the PE busy (and its p-state ramping toward full
    # clock) while the first input DMAs stream in. Results are never read.
    nc.vector.memset(warm[:], 0.0)

    def warmup(n):
        # Keep the PE continuously busy (matmul p-state stays ramped) while
        # input DMAs stream in; results are never read. Priority order lets
        # real work preempt the remaining warmups the moment it is ready.
        for w in range(n):
            pw = psum.tile([128, 2, 128], F32, tag="sc", bufs=2,
                           name=f"warm_{warmup.i}")
            warmup.i += 1
            nc.tensor.matmul(pw[:, 0], warm[:], warm[:])
            nc.tensor.matmul(pw[:, 1], warm[:], warm[:])
    warmup.i = 0

    # PE preamble: narrow K chunks and Q proj(0) interleaved with arrival
    # order; V proj chains are deferred into the qc0 filler stream. Warmups
    # interleave so the PE never idles (a PE idle gap resets the clock ramp
    # and the next chain would dispatch at the cold rate).
    warmup(25)
    k_proj(0)
    warmup(34)
    q_proj_one(0, q_ch0, 0)
    k_proj(1)
    for f in range(1, 4):
        q_proj_one(0, q_ch0, f)
    for c in range(2, 5):
        k_proj(c)

    # ---------------- pipelined attention + output projection ----------------
    # Score-pair emissions are the clock; between pairs we drain ~PAIR_NS of
    # deferred PE micro-tasks (ctx chains, transposes, out-proj chunks,
    # quarter Q-proj chains, V-proj chains) from a FIFO.
    from collections import deque
    fillers = deque()   # (cost_ns, thunk, tag)
    PAIR_NS = 700.0

    def drain(budget):
        got = 0.0
        while fillers and got < budget:
            cost, thunk, _ = fillers.popleft()
            thunk()
            got += cost

    def drain_tag(tag):
        """Emit (FIFO) until no filler with `tag` remains — order barrier."""
        while any(t == tag for _, _, t in fillers):
            cost, thunk, _ = fillers.popleft()
            thunk()

    def pair(qc, hd, kt2, expt):
        u, f = hd // 4, hd % 4
        lo, hi = u * DH, (u + 1) * DH
        ps = psum.tile([128, 2, 512], F32, tag="sc", bufs=2,
                       name=f"ps_{qc}_{hd}_{kt2}")
        for j in range(2):
            kt = 2 * kt2 + j
            nc.tensor.matmul(
                ps[:, j, :],
                ktd[lo:hi, kt * 128:(kt + 1) * 128],
                qtp[lo:hi, f, qc * 512:(qc + 1) * 512])
        nc.scalar.activation(
            expt[:, 2 * kt2:2 * kt2 + 2, :], ps[:], AF.Exp, scale=SCALE)

    # cn tiles buffer the normalized ctx of BOTH kv halves of an ftile
    # ([128 q, 128] = head f in cols 0:64, head f+4 in 64:128) so a single
    # XBAR DMA transpose produces the full-partition ctxT block — no PE
    # transpose, no DVE psum->sbuf copy.
    cns = {}

    def ctx_sub(qc, hd, sub, expt, cT):
        u, f = hd // 4, hd % 4

        def chain():
            pc = psum.tile([128, DH + 1], F32, tag="cx", bufs=2,
                           name=f"pc_{qc}_{hd}_{sub}")
            for kt in range(NKT):
                nc.tensor.matmul(
                    pc[:], expt[:, kt, sub * 128:(sub + 1) * 128],
                    vsb[:, kt, u * 66:u * 66 + DH + 1],
                    start=(kt == 0), stop=(kt == NKT - 1))
            rc = smal.tile([128, 1], F32, tag="rc", name=f"rc_{qc}_{hd}_{sub}")
            nc.vector.reciprocal(rc[:], pc[:, DH:DH + 1])
            if (qc, f, sub) not in cns:
                cns[(qc, f, sub)] = smal.tile(
                    [128, 128], BF16, tag="cn", bufs=20,
                    name=f"cn_{qc}_{f}_{sub}")
            cn = cns[(qc, f, sub)]
            nc.vector.tensor_scalar_mul(
                cn[:, u * DH:(u + 1) * DH], pc[:, 0:DH], rc[:])
            if u == 1:
                del cns[(qc, f, sub)]

                def transp():
                    nc.sync.dma_start_transpose(
                        cT[:, f, sub * 128:(sub + 1) * 128], cn[:])

                # near the FRONT (slightly deferred): must stay ahead of any
                # out-proj filler that reads this ctxT subtile.
                fillers.insert(min(2, len(fillers)),
                               (60.0, transp, f"ctx{qc}"))

        return chain

    def out_chunk(qc, sub, ch, cT, ost, tail=False):
        # tail chunks also rotate through the cx tag (idle after the last
        # ctx chain) for a 4-deep psum rotation at the end
        po = psum.tile([128, 512], F32,
                       tag=("cx" if tail and ch % 2 else "mm"), bufs=2,
                       name=f"po_{qc}_{sub}_{ch}")
        for i in range(4):
            nc.tensor.matmul(
                po[:], cT[:, i, sub * 128:(sub + 1) * 128],
                wo_sb[:, i, ch * 512:(ch + 1) * 512],
                start=(i == 0), stop=(i == 3))
        if tail and ch % 2 == 0:
            nc.scalar.copy(ost[:, ch * 512:(ch + 1) * 512], po[:])
        else:
            nc.vector.tensor_copy(ost[:, ch * 512:(ch + 1) * 512], po[:])
        rows = slice(qc * 512 + sub * 128, qc * 512 + (sub + 1) * 128)
        if tail:
            cols = slice(ch * 512, (ch + 1) * 512)
            nc.sync.dma_start(out[rows, cols], ost[:, cols])
        elif ch % 2 == 1:
            cols = slice((ch - 1) * 512, (ch + 1) * 512)
            nc.sync.dma_start(out[rows, cols], ost[:, cols])

    prev = None            # (qc, ctxT) pending output projection
    pend = []              # [(qc, hd, expt, ctxT)] ctx not yet enqueued
    # out-proj chunks flow through a global FIFO, drained unevenly across
    # the later qcs (qc3 has no next-qc Q-proj fillers, so it gets more).
    outq = deque()
    osts = {}

    def push_out_chunks(pqc, pcT):
        for sub in range(4):
            for ch in range(4):
                def mk(s=sub, c=ch, t=pcT, q=pqc):
                    if (q, s) not in osts:
                        osts[(q, s)] = ctxp.tile(
                            [128, DM], BF16, tag="ost", bufs=2,
                            name=f"ost_{q}_{s}")
                    out_chunk(q, s, c, t, osts[(q, s)])
                outq.append(mk)

    OUT_PER_HEAD = {
        1: [2, 1, 2, 1, 2, 1, 2, 1],
        2: [2, 1, 2, 1, 2, 1, 2, 1],
        3: [2, 2, 3, 3, 3, 3, 4, 4],
    }

    def warm_fill(n, tag):
        # idle-filler matmuls in Act-starved stretches: keep the PE clock
        # ramped so the next real chain dispatches at the fast rate
        for _ in range(n):
            def wf():
                pw = psum.tile([128, DH + 1], F32, tag="cx", bufs=2,
                               name=f"warmf_{warmup.i}")
                warmup.i += 1
                nc.tensor.matmul(pw[:], warm[:], warm[:, 0:DH + 1])
            fillers.append((110.0, wf, tag))

    for qc in range(4):
        ctxT = ctxp.tile([128, 4, 512], BF16, tag="ctxT", name=f"ctxT_{qc}")
        defer = 3 if qc < 3 else 1
        # stream next q chunk (and wo after the last input chunk)
        q_next = dma_q(qc + 1) if qc < 3 else None
        if qc == 0:
            nc.sync.dma_start(wo_sb[:], wo.rearrange("(i p) d -> p i d", p=128))
        # order barrier: qtp(qc) writes must be emitted before qc's pairs
        if qc > 0:
            drain_tag(f"qp{qc}")
        for hd in range(8):
            if qc == 0 and hd < 4:
                for t in range(4 * hd, 4 * hd + 4):
                    fillers.append((860.0, lambda t=t: v_proj(t), "vp"))
            # at qc start flush ALL pending ctx (so out fillers of the
            # previous qc are enqueued after every ctx write of that qc)
            want = 0 if hd == 0 else defer
            while len(pend) > want:
                pq2, phd, pexp, pcT2 = pend.pop(0)
                for sub in range(4):
                    fillers.append(
                        (440.0, ctx_sub(pq2, phd, sub, pexp, pcT2),
                         f"ctx{pq2}"))
            if hd == 0 and prev is not None:
                push_out_chunks(prev[0], prev[1])
            if qc > 0:
                for _ in range(OUT_PER_HEAD[qc][hd]):
                    if outq:
                        fillers.append((860.0, outq.popleft(), "out"))

            if qc < 3 and hd >= 4:
                fillers.append(
                    (3400.0, lambda f=hd - 4, qn=q_next, qc1=qc + 1: (
                        q_proj_one(qc1, qn, f)), f"qp{qc + 1}"))
            expt = expp.tile([128, NKT, 512], BF16, tag="exp",
                             name=f"exp_{qc}_{hd}")
            for kt2 in range(NKT // 2):
                pair(qc, hd, kt2, expt)
                drain(PAIR_NS)
            pend.append((qc, hd, expt, ctxT))
        prev = (qc, ctxT)

    # tail: remaining ctx heads, fillers, final output projection
    while pend:
        pq2, phd, pexp, pcT2 = pend.pop(0)
        for sub in range(4):
            ctx_sub(pq2, phd, sub, pexp, pcT2)()
            drain(PAIR_NS)
    drain(1e12)
    pqc, pcT = prev
    for sub in range(4):
        ost = ctxp.tile([128, DM], BF16, tag="ost", bufs=2,
                        name=f"ost_{pqc}_{sub}")
        for ch in range(4):
            out_chunk(pqc, sub, ch, pcT, ost, tail=True)


def _build():
    nc = bacc.Bacc("TRN2", target_bir_lowering=False, debug=False, num_devices=NC)
    qT = nc.dram_tensor("qT", [DM, S], BF16, kind="ExternalInput")
    kT = nc.dram_tensor("kT", [DM, S], BF16, kind="ExternalInput")
    vT = nc.dram_tensor("vT", [DM, S], BF16, kind="ExternalInput")
    # weights pre-arranged on host for contiguous-per-partition DMA
    wq = nc.dram_tensor("wq", [128, 4 * DT * 128], BF16, kind="ExternalInput")
    wk = nc.dram_tensor("wk", [128, DT * DKV], BF16, kind="ExternalInput")
    wv = nc.dram_tensor("wv", [128, DT * DKV], BF16, kind="ExternalInput")
    wo = nc.dram_tensor("wo", [DQ, DM], BF16, kind="ExternalInput")
    out = nc.dram_tensor("out", [S, DM], BF16, kind="ExternalOutput")
    with tile.TileContext(nc) as tc:
        with ExitStack() as ctx:
            _emit(ctx, tc, qT.ap(), kT.ap(), vT.ap(), wq.ap(), wk.ap(),
                  wv.ap(), wo.ap(), out.ap())
    nc.compile()
    return nc


def _make_runner(nc, n_cores=NC):
    """Build the sharded jit callable once; reuse across kernel() calls."""
    bass2jax.install_neuronx_cc_hook()
    partition_name = nc.partition_id_tensor.name if nc.partition_id_tensor else None
    in_names, out_names, out_avals, zero_outs = [], [], [], []
    for alloc in nc.m.functions[0].allocations:
        if not isinstance(alloc, mybir.MemoryLocationSet):
            continue
        name = alloc.memorylocations[0].name
        if alloc.kind == "ExternalInput":
            if name != partition_name:
                in_names.append(name)
        elif alloc.kind == "ExternalOutput":
            out_names.append(name)
            shape = tuple(alloc.tensor_shape)
            dtype = mybir.dt.np(alloc.dtype)
            out_avals.append(jax.core.ShapedArray(shape, dtype))
            zero_outs.append(np.zeros(shape, dtype))
    n_params = len(in_names)
    n_outs = len(out_avals)
    in_names_all = in_names + out_names
    if partition_name is not None:
        in_names_all.append(partition_name)
    donate = tuple(range(n_params, n_params + n_outs))

    def _body(*args):
        operands = list(args)
        if partition_name is not None:
            operands.append(bass2jax.partition_id_tensor())
        outs = bass2jax._bass_exec_p.bind(
            *operands,
            out_avals=tuple(out_avals),
            in_names=tuple(in_names_all),
            out_names=tuple(out_names),
            lowering_input_output_aliases=(),
            sim_require_finite=True,
            sim_require_nnan=True,
            nc=nc,
        )
        return tuple(outs)

    devices = jax.devices()[:n_cores]
    mesh = Mesh(np.asarray(devices), ("core",))
    in_specs = (PartitionSpec("core"),) * (n_params + n_outs)
    out_specs = (PartitionSpec("core"),) * len(out_names)
    sharded = jax.jit(
        shard_map(_body, mesh=mesh, in_specs=in_specs, out_specs=out_specs,
                  check_rep=False),
        donate_argnums=donate, keep_unused=True)
    sh = NamedSharding(mesh, PartitionSpec("core"))
    return sharded, in_names, out_names, zero_outs, sh


def _run(in_maps):
    if "nc" not in _cache:
        _cache["nc"] = _build()
    if "runner" not in _cache:
        _cache["runner"] = _make_runner(_cache["nc"])
    sharded, in_names, out_names, zero_outs, sh = _cache["runner"]
    n = NC
    concat_in = [
        jax.device_put(
            np.concatenate([np.asarray(in_maps[c][nm]) for c in range(n)], 0), sh)
        for nm in in_names
    ]
    zeros = [
        jax.device_put(np.zeros((n * z.shape[0], *z.shape[1:]), z.dtype), sh)
        for z in zero_outs
    ]
    outs = sharded(*concat_in, *zeros)
    i = out_names.index("out")
    arr = np.asarray(outs[i])           # [NC*S, DM]
    return arr.reshape(n, S, DM)


def kernel(q, k, v, Wq, Wk, Wv, Wo):
    q = np.asarray(q, dtype=np.float32)
    k = np.asarray(k, dtype=np.float32)
    v = np.asarray(v, dtype=np.float32)
    bf = ml_dtypes.bfloat16
    qTh = np.ascontiguousarray(q.astype(bf).transpose(0, 2, 1))
    kTh = np.ascontiguousarray(k.astype(bf).transpose(0, 2, 1))
    vTh = np.ascontiguousarray(v.astype(bf).transpose(0, 2, 1))
    Wq32 = np.asarray(Wq, dtype=np.float32)
    Wk32 = np.asarray(Wk, dtype=np.float32)
    Wv32 = np.asarray(Wv, dtype=np.float32)
    Wo32 = np.asarray(Wo, dtype=np.float32)

    in_maps = []
    for c in range(NC):
        b, g = c // 4, c % 4
        # head (u, f) -> qtp ftile f, rows u*64:(u+1)*64
        wq_s = Wq32[:, g * DQ:(g + 1) * DQ].reshape(DM, 2, 4, DH)
        wq_p = wq_s.transpose(0, 2, 1, 3).reshape(DM, DQ)
        # device layout [p, f, dt, d]: wq_pre[p, f*DT*128 + dt*128 + x]
        # = wq_p[dt*128 + p, f*128 + x]
        wq_pre = (wq_p.reshape(DT, 128, 4, 128)
                  .transpose(1, 2, 0, 3).reshape(128, 4 * DT * 128))
        # device layout [p, dt, m]: w_pre[p, dt*DKV + m] = w[dt*128 + p, m]
        wk_s = Wk32[:, g * DKV:(g + 1) * DKV]
        wk_pre = wk_s.reshape(DT, 128, DKV).transpose(1, 0, 2).reshape(128, -1)
        wv_s = Wv32[:, g * DKV:(g + 1) * DKV]
        wv_pre = wv_s.reshape(DT, 128, DKV).transpose(1, 0, 2).reshape(128, -1)
        wo_s = Wo32[g * DQ:(g + 1) * DQ, :].reshape(2, 4, DH, DM)
        wo_p = wo_s.transpose(1, 0, 2, 3).reshape(DQ, DM)
        in_maps.append({
            "qT": qTh[b], "kT": kTh[b], "vT": vTh[b],
            "wq": np.ascontiguousarray(wq_pre.astype(bf)),
            "wk": np.ascontiguousarray(wk_pre.astype(bf)),
            "wv": np.ascontiguousarray(wv_pre.astype(bf)),
            "wo": np.ascontiguousarray(wo_p.astype(bf)),
        })
    partials = _run(in_maps)            # [8, S, DM] bf16
    out = partials.astype(np.float32, copy=False).reshape(2, 4, S, DM).sum(axis=1)
    return out
